# revision 15
# baseline (speedup 1.0000x reference)
"""Hyperbolic GNN classifier on 8 Trainium2 NeuronCores (Bass/Tile), v2.

Node-sharded (8192 nodes/core). Per layer: local transform -> fp16 table
(rows: 256 feats + ||row||^2 at col 256, padded to 384 elems = 768B) ->
chunked AllGather -> 16-step Mobius left-fold using batched dma_gather
(int16 indices biased by -32768 against a base offset +32768 rows; 4
SWDGE queues emit descriptors on 4 Q7 core pairs in parallel).

Fold state per row: stored direction A (fp16) with scalars (alpha, nA2):
point x = alpha*A, nA2 = ||A||^2. Per step: per-tile STT dot accumulate
d = <A,y>, fp32 scalar chain on [128, G] tiles, per-tile STT axpy
A += beta*y. All transcendentals are even-function polynomials in
squared norms (atanh(u^.5)/u^.5, tanh(v^.5)/v^.5) evaluated on DVE --
the scalar engine only ever runs Square (one ACT table set, no reloads).
"""

import os

import numpy as np

import concourse.bass as bass
import concourse.bacc as bacc
import concourse.mybir as mybir
import concourse.tile as tile
from concourse.bass_utils import run_bass_kernel_spmd
from concourse.masks import make_identity
from concourse.tile_rust import add_dep_helper

f32 = mybir.dt.float32
f16 = mybir.dt.float16
bf16 = mybir.dt.bfloat16
i32 = mybir.dt.int32
i16 = mybir.dt.int16
OP = mybir.AluOpType
AF = mybir.ActivationFunctionType

NCORES = 8
N = 65536
KDEG = 16
DIN = 512
DH = 256
DOUT = 64
P = 128
NSH = N // NCORES            # 8192 nodes per core
TILES = NSH // P             # 64 tiles per core
SE = 384                     # table row elems fp16 (768B); col 256 = y2
G = 32                       # tiles per fold group
NG = TILES // G              # 2 groups
NQ = 4                       # SWDGE queues
NCALL = 4                    # gather calls per group-step
TPQ = G // NCALL             # tiles per gather call (8)
NI = TPQ * P + 16            # idxs per call incl 16-pad (1040)
IB = (NI + 15) // 16         # idx cols actually read per call (65)
IBS = 80                     # idx block stride in cols (160B, 32B-aligned)
NCH = 4                      # collective chunks per layer (16 tiles each)
CHROWS = NSH // NCH          # 2048 shard rows per chunk
TG = 4                       # transform tiles per chain batch
NORM = float(KDEG) ** -0.5   # 0.25
NORM2 = 1.0 / KDEG

STAGE = int(os.environ.get("KSTAGE", "5"))
KDEBUG = bool(int(os.environ.get("KDEBUG", "0")))
TRACE = False
LAST_RESULT = None
LAST_EXEC_NS = None
_NC_CACHE = {}


def _poly_horner(nc, pool, u, coeffs, g, tag, p=P):
    """p(u) = 1 + u*(c1 + u*(c2 + ... )) on [p, g] fp32 DVE tiles.

    coeffs = [c1, c2, ...] low-order first. Returns the poly tile.
    """
    V = nc.vector
    cs = coeffs[::-1]  # high order first
    h = pool.tile([p, g], f32, tag=f"{tag}_h")
    if len(cs) == 1:
        V.tensor_scalar(out=h[:], in0=u[:], scalar1=cs[0], scalar2=1.0,
                        op0=OP.mult, op1=OP.add)
        return h
    V.tensor_scalar(out=h[:], in0=u[:], scalar1=cs[0], scalar2=cs[1],
                    op0=OP.mult, op1=OP.add)
    for c in cs[2:]:
        t = pool.tile([p, g], f32, tag=f"{tag}_t")
        V.tensor_tensor(out=t[:], in0=h[:], in1=u[:], op=OP.mult)
        h = pool.tile([p, g], f32, tag=f"{tag}_h")
        V.tensor_scalar(out=h[:], in0=t[:], scalar1=1.0, scalar2=c,
                        op0=OP.mult, op1=OP.add)
    t = pool.tile([p, g], f32, tag=f"{tag}_t")
    V.tensor_tensor(out=t[:], in0=h[:], in1=u[:], op=OP.mult)
    h = pool.tile([p, g], f32, tag=f"{tag}_hf")
    V.tensor_scalar(out=h[:], in0=t[:], scalar1=1.0, scalar2=1.0,
                    op0=OP.mult, op1=OP.add)
    return h


# atanh(sqrt(u))/sqrt(u) = 1 + u/3 + u^2/5 + u^3/7 + u^4/9
A_COEF = [1.0 / 3, 1.0 / 5, 1.0 / 7, 1.0 / 9]
# tanh(sqrt(v))/sqrt(v) = 1 - v/3 + 2v^2/15 - 17v^3/315
T_COEF = [-1.0 / 3, 2.0 / 15, -17.0 / 315]


def _scaled(coeffs, s):
    return [c * (s ** (i + 1)) for i, c in enumerate(coeffs)]


def _fold_chain(nc, ch, S2, y2, alpha, nA2, g):
    """Mobius fold scalars: given S2=2<A,y>, y2, state (alpha,nA2) ->
    (beta, alpha', nA2'). All [P, g] fp32."""
    V = nc.vector
    t1 = ch.tile([P, g], f32, tag="fc_t1")  # 2 a d
    V.tensor_tensor(out=t1[:], in0=alpha[:], in1=S2[:], op=OP.mult)
    u1 = ch.tile([P, g], f32, tag="fc_u1")  # 2ad + y2
    V.tensor_tensor(out=u1[:], in0=t1[:], in1=y2[:], op=OP.add)
    aa = ch.tile([P, g], f32, tag="fc_aa")
    V.tensor_tensor(out=aa[:], in0=alpha[:], in1=alpha[:], op=OP.mult)
    a2 = ch.tile([P, g], f32, tag="fc_a2")
    V.tensor_tensor(out=a2[:], in0=aa[:], in1=nA2[:], op=OP.mult)
    t2 = ch.tile([P, g], f32, tag="fc_t2")
    V.tensor_tensor(out=t2[:], in0=a2[:], in1=y2[:], op=OP.mult)
    u2 = ch.tile([P, g], f32, tag="fc_u2")  # 2ad + a2*y2
    V.tensor_tensor(out=u2[:], in0=t1[:], in1=t2[:], op=OP.add)
    den = ch.tile([P, g], f32, tag="fc_den")  # 1 + 2ad + a2*y2
    V.tensor_scalar(out=den[:], in0=u2[:], scalar1=1.0, scalar2=None,
                    op0=OP.add)
    ca = ch.tile([P, g], f32, tag="fc_ca")   # 1 + 2ad + y2
    V.tensor_scalar(out=ca[:], in0=u1[:], scalar1=1.0, scalar2=None,
                    op0=OP.add)
    r = ch.tile([P, g], f32, tag="fc_r")
    V.reciprocal(r[:], den[:])
    t3 = ch.tile([P, g], f32, tag="fc_t3")   # alpha*ca
    V.tensor_tensor(out=t3[:], in0=alpha[:], in1=ca[:], op=OP.mult)
    alpha_n = ch.tile([P, g], f32, tag="fc_an")
    V.tensor_tensor(out=alpha_n[:], in0=t3[:], in1=r[:], op=OP.mult)
    w = ch.tile([P, g], f32, tag="fc_w")     # 1 - a2
    V.tensor_scalar(out=w[:], in0=a2[:], scalar1=-1.0, scalar2=1.0,
                    op0=OP.mult, op1=OP.add)
    vinv = ch.tile([P, g], f32, tag="fc_vi")
    V.reciprocal(vinv[:], t3[:])
    beta = ch.tile([P, g], f32, tag="fc_be")
    V.tensor_tensor(out=beta[:], in0=w[:], in1=vinv[:], op=OP.mult)
    t4 = ch.tile([P, g], f32, tag="fc_t4")  # beta * 2d
    V.tensor_tensor(out=t4[:], in0=beta[:], in1=S2[:], op=OP.mult)
    t5 = ch.tile([P, g], f32, tag="fc_t5")   # nA2 + 2 beta d
    V.tensor_tensor(out=t5[:], in0=t4[:], in1=nA2[:], op=OP.add)
    t6 = ch.tile([P, g], f32, tag="fc_t6")
    V.tensor_tensor(out=t6[:], in0=beta[:], in1=beta[:], op=OP.mult)
    t7 = ch.tile([P, g], f32, tag="fc_t7")
    V.tensor_tensor(out=t7[:], in0=t6[:], in1=y2[:], op=OP.mult)
    nA2_n = ch.tile([P, g], f32, tag="fc_nn")
    V.tensor_tensor(out=nA2_n[:], in0=t5[:], in1=t7[:], op=OP.add)
    return beta, alpha_n, nA2_n


def _acc_row(acc, t):
    return acc[:, t // TPQ, t % TPQ, 0:DH]


def _build_nc():
    nc = bacc.Bacc("TRN2", target_bir_lowering=False, debug=False,
                   num_devices=NCORES, num_swdge_queues=NQ)
    featT = nc.dram_tensor("featT", [DIN, NSH], bf16, kind="ExternalInput")
    featR = nc.dram_tensor("featR", [NSH, DIN], bf16, kind="ExternalInput")
    idx = nc.dram_tensor("idx", [P, NG * KDEG * NCALL * IBS], i16,
                         kind="ExternalInput")
    sel = nc.dram_tensor("sel", [8, 1], i32, kind="ExternalInput")
    W1 = nc.dram_tensor("W1", [DIN, DH], bf16, kind="ExternalInput")
    b1 = nc.dram_tensor("b1", [1, DH], f16, kind="ExternalInput")
    W2 = nc.dram_tensor("W2", [DH, DH], f16, kind="ExternalInput")
    b2 = nc.dram_tensor("b2", [1, DH], f16, kind="ExternalInput")
    WlT = nc.dram_tensor("WlT", [DH, DOUT], f16, kind="ExternalInput")
    bl = nc.dram_tensor("bl", [1, DOUT], f32, kind="ExternalInput")
    out = nc.dram_tensor("out", [8, DOUT], f32, kind="ExternalOutput")
    if KDEBUG:
        dbg_t1 = nc.dram_tensor("dbg_t1", [NSH, SE], f16,
                                kind="ExternalOutput")
        dbg_h1 = nc.dram_tensor("dbg_h1", [NSH, SE], f16,
                                kind="ExternalOutput")
        dbg_h2 = nc.dram_tensor("dbg_h2", [NSH, SE], f16,
                                kind="ExternalOutput")

    with tile.TileContext(nc) as tc:
        with (
            tc.tile_pool(name="wt", bufs=1) as wt,
            tc.tile_pool(name="sb", bufs=3) as sb,
            tc.tile_pool(name="ch", bufs=2) as ch,
            tc.tile_pool(name="yb", bufs=2) as yb,
            tc.tile_pool(name="ac", bufs=1) as ac,
            tc.tile_pool(name="df", bufs=2) as df,
            tc.tile_pool(name="tp", bufs=1) as tp,
            tc.tile_pool(name="ps", bufs=2, space="PSUM") as ps,
            tc.tile_pool(name="psmx", bufs=4, space="PSUM") as psmx,
            tc.tile_pool(name="psm2", bufs=2, space="PSUM") as psm2,
            tc.tile_pool(name="dr", bufs=1, space="DRAM") as dr,
        ):
            ident = wt.tile([P, P], f16, tag="ident")
            make_identity(nc, ident[:])

            # ---- weights ----
            W1sb = wt.tile([P, DIN // P, DH], bf16, tag="W1sb")
            nc.sync.dma_start(W1sb[:],
                              W1[:].rearrange("(a p) d -> p a d", p=P))
            W2sb = wt.tile([P, DH // P, DH], f16, tag="W2sb")
            nc.sync.dma_start(W2sb[:],
                              W2[:].rearrange("(a p) d -> p a d", p=P))
            Wlsb = wt.tile([P, DH // P, DOUT], f16, tag="Wlsb")
            nc.sync.dma_start(Wlsb[:],
                              WlT[:].rearrange("(a p) d -> p a d", p=P))
            b1row = wt.tile([1, DH], f16, tag="b1row")
            nc.sync.dma_start(b1row[:], b1[:])
            b1b = wt.tile([P, DH], f16, tag="b1b")
            nc.gpsimd.partition_broadcast(b1b[:], b1row[:])
            b2row = wt.tile([1, DH], f16, tag="b2row")
            nc.sync.dma_start(b2row[:], b2[:])
            b2b = wt.tile([P, DH], f16, tag="b2b")
            nc.gpsimd.partition_broadcast(b2b[:], b2row[:])
            blrow = wt.tile([1, DOUT], f32, tag="blrow")
            nc.sync.dma_start(blrow[:], bl[:])
            blb = wt.tile([8, DOUT], f32, tag="blb")
            nc.gpsimd.partition_broadcast(blb[:], blrow[:], channels=8)
            bscr = wt.tile([P, DH], f16, tag="bscr")
            b1n2 = wt.tile([P, 1], f32, tag="b1n2")
            nc.scalar.activation(bscr[:], b1b[:], AF.Square, accum_out=b1n2[:])
            bscr2 = wt.tile([P, DH], f16, tag="bscr2")
            b2n2 = wt.tile([P, 1], f32, tag="b2n2")
            nc.scalar.activation(bscr2[:], b2b[:], AF.Square,
                                 accum_out=b2n2[:])
            bscr3 = wt.tile([8, DOUT], f32, tag="bscr3")
            bln2 = wt.tile([8, 1], f32, tag="bln2")
            nc.scalar.activation(bscr3[:], blb[:], AF.Square,
                                 accum_out=bln2[:])

            # ---- indices (biased int16, replicated over partitions) ----
            idx_sb = wt.tile([P, NG * KDEG * NCALL * IBS], i16, tag="idx")
            nc.sync.dma_start(idx_sb[:], idx[:])

            def idx_slice(gi, k, j):
                o = (((gi * KDEG) + k) * NCALL + j) * IBS
                return idx_sb[:, o : o + IB]

            # ---- DRAM tables ----
            t1sh = dr.tile([NSH, SE], f16, tag="t1sh")
            t1full = dr.tile([N, SE], f16, tag="t1full", addr_space="Shared")
            h1sh = dr.tile([NSH, SE], f16, tag="h1sh")
            h1full = dr.tile([N, SE], f16, tag="h1full", addr_space="Shared")
            h2sh = dr.tile([NSH, SE], f16, tag="h2sh")

            # ============ Phase T: layer-1 transform ============
            cc_insts_1 = []
            for bt in (range(0, TILES, TG) if STAGE >= 1 else []):
                xn2 = ch.tile([P, TG], f32, tag="tf_xn2")
                mxn2 = ch.tile([P, TG], f32, tag="tf_mxn2")
                pmx_list = []
                for j in range(TG):
                    t = bt + j
                    ftT = sb.tile([P, DIN // P, P], bf16, tag="ftT")
                    nc.sync.dma_start(
                        ftT[:],
                        featT[:, t * P : (t + 1) * P].rearrange(
                            "(c p) n -> p c n", p=P))
                    ftR = sb.tile([P, DIN], bf16, tag="ftR")
                    nc.sync.dma_start(ftR[:], featR[t * P : (t + 1) * P, :])
                    sq = sb.tile([P, DIN], bf16, tag="sq")
                    nc.scalar.activation(sq[:], ftR[:], AF.Square,
                                         accum_out=xn2[:, j : j + 1])
                    pmx = psmx.tile([P, DH], f32, tag="pmx")
                    for c in range(DIN // P):
                        nc.tensor.matmul(out=pmx[:], lhsT=ftT[:, c, :],
                                         rhs=W1sb[:, c, :],
                                         start=(c == 0),
                                         stop=(c == DIN // P - 1))
                    msq = sb.tile([P, DH], f16, tag="msq")
                    nc.scalar.activation(msq[:], pmx[:], AF.Square,
                                         accum_out=mxn2[:, j : j + 1])
                    pmx_list.append(pmx)
                # batched scalar chain: s=poly_a(xn2/16), r2=(mxn2/16)*s^2
                s = _poly_horner(nc, ch, xn2, _scaled(A_COEF, NORM2), TG,
                                 "tfa")
                ssq = ch.tile([P, TG], f32, tag="tf_ssq")
                nc.vector.tensor_tensor(out=ssq[:], in0=s[:], in1=s[:],
                                        op=OP.mult)
                mxs = ch.tile([P, TG], f32, tag="tf_mxs")
                nc.vector.tensor_tensor(out=mxs[:], in0=mxn2[:], in1=ssq[:],
                                        op=OP.mult)
                pt = _poly_horner(nc, ch, mxs, _scaled(T_COEF, NORM2), TG,
                                  "tft")
                spt = ch.tile([P, TG], f32, tag="tf_spt")
                nc.vector.tensor_tensor(out=spt[:], in0=s[:], in1=pt[:],
                                        op=OP.mult)
                coef = ch.tile([P, TG], f32, tag="tf_coef")
                nc.vector.tensor_scalar(out=coef[:], in0=spt[:],
                                        scalar1=NORM, scalar2=None,
                                        op0=OP.mult)
                # y2 = (r*pt)^2 = (mxs/16)*pt^2 * s^2... careful:
                # row = coef*pmx; y2 = coef^2 * mxn2
                c2 = ch.tile([P, TG], f32, tag="tf_c2")
                nc.vector.tensor_tensor(out=c2[:], in0=coef[:], in1=coef[:],
                                        op=OP.mult)
                y2r = ch.tile([P, TG], f32, tag="tf_y2r")
                nc.vector.tensor_tensor(out=y2r[:], in0=c2[:], in1=mxn2[:],
                                        op=OP.mult)
                for j in range(TG):
                    t = bt + j
                    htile = sb.tile([P, 257], f16, tag="htile")
                    nc.vector.tensor_scalar(
                        out=htile[:, 0:DH], in0=pmx_list[j][:],
                        scalar1=coef[:, j : j + 1], scalar2=None, op0=OP.mult)
                    nc.vector.tensor_copy(out=htile[:, DH : DH + 1],
                                          in_=y2r[:, j : j + 1])
                    nc.sync.dma_start(t1sh[t * P : (t + 1) * P, 0:257],
                                      htile[:])

            if STAGE >= 2:
                cc = nc.gpsimd.collective_compute(
                    "AllGather", OP.bypass,
                    replica_groups=[list(range(NCORES))],
                    ins=[t1sh[:]], outs=[t1full[:]])
                cc_insts_1.append(cc)

            # ============ fold machinery ============
            def fold_layer(table, cc_insts, bias_b, bias_n2, matvec2,
                           dst_sh):
                """One aggregation layer over `table` (gather source).

                matvec2: if True apply W2 mobius matvec after the fold
                (layer 2); else layer-1 style (store with NORM scale).
                """
                base = table[N // 2 :, :]
                for gi in range(NG):
                    acc = ac.tile([P, NCALL, TPQ + 1, SE], f16, tag="acc")
                    gaths = []
                    for j in range(NCALL):
                        g0 = nc.gpsimd.dma_gather(
                            out_ap=acc[:, j, :, :], in_ap=base,
                            idxs_ap=idx_slice(gi, 0, j),
                            num_idxs=NI, num_idxs_reg=NI, elem_size=SE,
                            single_packet=False, queue_num=j % NQ)
                        gaths.append(g0)
                    alpha = ch.tile([P, G], f32, tag="st_alpha")
                    nc.vector.memset(alpha[:], 1.0)
                    nA2 = ch.tile([P, G], f32, tag="st_nA2")
                    nc.vector.tensor_copy(
                        out=nA2[:].rearrange("p (q b) -> p q b", q=NCALL),
                        in_=acc[:, :, 0:TPQ, DH])
                    scratch = sb.tile([P, DH], f16, tag="dotscr")

                    for k in range(1, KDEG):
                        ybuf = yb.tile([P, NCALL, TPQ + 1, SE], f16,
                                       tag="ybuf")
                        for j in range(NCALL):
                            gk = nc.gpsimd.dma_gather(
                                out_ap=ybuf[:, j, :, :], in_ap=base,
                                idxs_ap=idx_slice(gi, k, j),
                                num_idxs=NI, num_idxs_reg=NI, elem_size=SE,
                                single_packet=False, queue_num=j % NQ)
                            gaths.append(gk)
                        prod = df.tile([P, NCALL, TPQ, DH], bf16, tag="prod")
                        nc.vector.tensor_tensor(
                            out=prod[:], in0=acc[:, :, 0:TPQ, 0:DH],
                            in1=ybuf[:, :, 0:TPQ, 0:DH], op=OP.mult)
                        red = df.tile([P, NCALL, TPQ, DH // 2], bf16,
                                      tag="red")
                        nc.vector.tensor_tensor(
                            out=red[:], in0=prod[:, :, :, 0 : DH // 2],
                            in1=prod[:, :, :, DH // 2 : DH], op=OP.add)
                        w = DH // 2
                        while w > 4:
                            nc.vector.tensor_tensor(
                                out=red[:, :, :, 0 : w // 2],
                                in0=red[:, :, :, 0 : w // 2],
                                in1=red[:, :, :, w // 2 : w], op=OP.add)
                            w //= 2
                        d = ch.tile([P, G], f32, tag="fd_d")
                        nc.vector.tensor_reduce(
                            out=d[:].rearrange("p (q b) -> p q b", q=NCALL),
                            in_=red[:, :, :, 0:4], axis=mybir.AxisListType.X,
                            op=OP.add)
                        y2 = ch.tile([P, G], f32, tag="fd_y2")
                        nc.vector.tensor_copy(
                            out=y2[:].rearrange("p (q b) -> p q b", q=NCALL),
                            in_=ybuf[:, :, 0:TPQ, DH])
                        S2 = ch.tile([P, G], f32, tag="fd_S2")
                        nc.vector.tensor_scalar(out=S2[:], in0=d[:],
                                                scalar1=2.0, scalar2=None,
                                                op0=OP.mult)
                        beta, alpha, nA2 = _fold_chain(nc, ch, S2, y2,
                                                       alpha, nA2, G)
                        tmp = tp.tile([P, NCALL, TPQ, DH], f16, tag="tmp")
                        for t in range(G):
                            nc.scalar.activation(
                                tmp[:, t // TPQ, t % TPQ, :],
                                ybuf[:, t // TPQ, t % TPQ, 0:DH],
                                AF.Copy, scale=beta[:, t : t + 1])
                        nc.vector.tensor_tensor(
                            out=acc[:, :, 0:TPQ, 0:DH],
                            in0=acc[:, :, 0:TPQ, 0:DH], in1=tmp[:],
                            op=OP.add)
                    for gth in gaths:
                        for cci in cc_insts:
                            add_dep_helper(gth.ins, cci.ins,
                                           reason="gather after allgather")

                    if matvec2:
                        # x = alpha*A; mx = alpha*(A @ W2); v-rows = pmA
                        a2 = ch.tile([P, G], f32, tag="mv_a2")
                        nc.vector.tensor_tensor(out=a2[:], in0=alpha[:],
                                                in1=alpha[:], op=OP.mult)
                        nc.vector.tensor_tensor(out=a2[:], in0=a2[:],
                                                in1=nA2[:], op=OP.mult)
                        s2g = _poly_horner(nc, ch, a2, A_COEF, G, "mva")
                        msqA = ch.tile([P, G], f32, tag="mv_msq")
                        for t in range(G):
                            aT = sb.tile([P, DH // P, P], f16, tag="aT")
                            for c in range(DH // P):
                                ptr = ps.tile([P, P], f16, tag="ptr")
                                nc.tensor.transpose(
                                    out=ptr[:],
                                    in_=acc[:, t // TPQ, t % TPQ,
                                            c * P : (c + 1) * P],
                                    identity=ident[:])
                                nc.vector.tensor_copy(out=aT[:, c, :],
                                                      in_=ptr[:])
                            pm2 = psm2.tile([P, DH], f32, tag="pm2")
                            for c in range(DH // P):
                                nc.tensor.matmul(out=pm2[:],
                                                 lhsT=aT[:, c, :],
                                                 rhs=W2sb[:, c, :],
                                                 start=(c == 0),
                                                 stop=(c == DH // P - 1))
                            ms2 = sb.tile([P, DH], f16, tag="ms2")
                            nc.scalar.activation(ms2[:], pm2[:], AF.Square,
                                                 accum_out=msqA[:, t : t + 1])
                            # overwrite acc row with pmA (fp16)
                            nc.vector.tensor_copy(
                                out=acc[:, t // TPQ, t % TPQ, 0:DH],
                                in_=pm2[:])
                        # r2 = a2 * msqA * s^2; coef = alpha*s*pt(r2)*NORM
                        ssq = ch.tile([P, G], f32, tag="mv_ssq")
                        nc.vector.tensor_tensor(out=ssq[:], in0=s2g[:],
                                                in1=s2g[:], op=OP.mult)
                        aa2 = ch.tile([P, G], f32, tag="mv_aa2")
                        nc.vector.tensor_tensor(out=aa2[:], in0=alpha[:],
                                                in1=alpha[:], op=OP.mult)
                        r2 = ch.tile([P, G], f32, tag="mv_r2")
                        nc.vector.tensor_tensor(out=r2[:], in0=aa2[:],
                                                in1=msqA[:], op=OP.mult)
                        nc.vector.tensor_tensor(out=r2[:], in0=r2[:],
                                                in1=ssq[:], op=OP.mult)
                        ptg = _poly_horner(nc, ch, r2, T_COEF, G, "mvt")
                        co = ch.tile([P, G], f32, tag="mv_co")
                        nc.vector.tensor_tensor(out=co[:], in0=alpha[:],
                                                in1=s2g[:], op=OP.mult)
                        nc.vector.tensor_tensor(out=co[:], in0=co[:],
                                                in1=ptg[:], op=OP.mult)
                        alpha = ch.tile([P, G], f32, tag="st_alpha")
                        nc.vector.tensor_scalar(out=alpha[:], in0=co[:],
                                                scalar1=NORM, scalar2=None,
                                                op0=OP.mult)
                        nA2 = msqA
                        bias_row, bias_sq = b2b, b2n2
                        scale = 1.0
                    else:
                        alpha_s = ch.tile([P, G], f32, tag="st_alpha")
                        nc.vector.tensor_scalar(out=alpha_s[:], in0=alpha[:],
                                                scalar1=NORM, scalar2=None,
                                                op0=OP.mult)
                        alpha = alpha_s
                        bias_row, bias_sq = bias_b, bias_n2
                        scale = NORM

                    # bias fold: y = bias_row (broadcast), y2 = bias_sq
                    d = ch.tile([P, G], f32, tag="fd_d")
                    for t in range(G):
                        nc.vector.scalar_tensor_tensor(
                            out=scratch[:], in0=_acc_row(acc, t), scalar=1.0,
                            in1=bias_row[:], op0=OP.mult, op1=OP.mult,
                            accum_out=d[:, t : t + 1])
                    S2b = ch.tile([P, G], f32, tag="fd_S2b")
                    nc.vector.tensor_scalar(out=S2b[:], in0=d[:], scalar1=2.0,
                                            scalar2=None, op0=OP.mult)
                    y2b = ch.tile([P, G], f32, tag="fd_y2b")
                    nc.vector.tensor_scalar(out=y2b[:], in0=d[:], scalar1=0.0,
                                            scalar2=bias_sq[:, 0:1],
                                            op0=OP.mult, op1=OP.add)
                    beta, alpha, nA2 = _fold_chain(nc, ch, S2b, y2b, alpha,
                                                   nA2, G)
                    for t in range(G):
                        nc.vector.scalar_tensor_tensor(
                            out=_acc_row(acc, t), in0=bias_row[:],
                            scalar=beta[:, t : t + 1], in1=_acc_row(acc, t),
                            op0=OP.mult, op1=OP.add)

                    # activation: logmap0 -> relu -> expmap0, store
                    rn2g = ch.tile([P, G], f32, tag="as_rn2")
                    for t in range(G):
                        zrow = _acc_row(acc, t)
                        nc.vector.tensor_scalar(out=zrow, in0=zrow,
                                                scalar1=0.0, scalar2=None,
                                                op0=OP.max)
                        zsq = sb.tile([P, DH], f16, tag="zsq")
                        nc.scalar.activation(zsq[:], zrow, AF.Square,
                                             accum_out=rn2g[:, t : t + 1])
                    a2f = ch.tile([P, G], f32, tag="as_a2f")
                    nc.vector.tensor_tensor(out=a2f[:], in0=alpha[:],
                                            in1=alpha[:], op=OP.mult)
                    nc.vector.tensor_tensor(out=a2f[:], in0=a2f[:],
                                            in1=nA2[:], op=OP.mult)
                    sg = _poly_horner(nc, ch, a2f, A_COEF, G, "asa")
                    c1 = ch.tile([P, G], f32, tag="as_c1")
                    nc.vector.tensor_tensor(out=c1[:], in0=sg[:],
                                            in1=alpha[:], op=OP.mult)
                    c1sq = ch.tile([P, G], f32, tag="as_c1sq")
                    nc.vector.tensor_tensor(out=c1sq[:], in0=c1[:],
                                            in1=c1[:], op=OP.mult)
                    vr2 = ch.tile([P, G], f32, tag="as_vr2")
                    nc.vector.tensor_tensor(out=vr2[:], in0=c1sq[:],
                                            in1=rn2g[:], op=OP.mult)
                    ptg2 = _poly_horner(nc, ch, vr2, T_COEF, G, "ast")
                    gam = ch.tile([P, G], f32, tag="as_gam")
                    nc.vector.tensor_tensor(out=gam[:], in0=c1[:],
                                            in1=ptg2[:], op=OP.mult)
                    if scale != 1.0:
                        gs = ch.tile([P, G], f32, tag="as_gs")
                        nc.vector.tensor_scalar(out=gs[:], in0=gam[:],
                                                scalar1=scale, scalar2=None,
                                                op0=OP.mult)
                        gam = gs
                    gg = ch.tile([P, G], f32, tag="as_gg")
                    nc.vector.tensor_tensor(out=gg[:], in0=gam[:],
                                            in1=gam[:], op=OP.mult)
                    y2o = ch.tile([P, G], f32, tag="as_y2o")
                    nc.vector.tensor_tensor(out=y2o[:], in0=gg[:],
                                            in1=rn2g[:], op=OP.mult)
                    for t in range(G):
                        ti = gi * G + t
                        htile = sb.tile([P, 257], f16, tag="htile")
                        nc.vector.tensor_scalar(
                            out=htile[:, 0:DH], in0=_acc_row(acc, t),
                            scalar1=gam[:, t : t + 1], scalar2=None,
                            op0=OP.mult)
                        nc.vector.tensor_copy(out=htile[:, DH : DH + 1],
                                              in_=y2o[:, t : t + 1])
                        nc.sync.dma_start(
                            dst_sh[ti * P : (ti + 1) * P, 0:257], htile[:])
                return

            # ============ Phase A1 + AllGather h1 ============
            cc_insts_2 = []
            if STAGE >= 3:
                fold_layer(t1full, cc_insts_1, b1b, b1n2, False, h1sh)
                if STAGE >= 4:
                    cc = nc.gpsimd.collective_compute(
                        "AllGather", OP.bypass,
                        replica_groups=[list(range(NCORES))],
                        ins=[h1sh[:]], outs=[h1full[:]])
                    cc_insts_2.append(cc)

            # ============ Phase A2 ============
            if STAGE >= 4:
                fold_layer(h1full, cc_insts_2, b2b, b2n2, True, h2sh)

            # ============ Phase D: classifier ============
            if STAGE >= 5:
                selt = wt.tile([8, 1], i32, tag="selt")
                nc.sync.dma_start(selt[:], sel[:])
                hr = wt.tile([8, SE], f16, tag="hr")
                nc.gpsimd.indirect_dma_start(
                    out=hr[:], out_offset=None, in_=h2sh[:],
                    in_offset=bass.IndirectOffsetOnAxis(ap=selt[:, 0:1],
                                                        axis=0))
                hT = wt.tile([P, DH // P, 8], f16, tag="hT")
                for c in range(DH // P):
                    pt3 = ps.tile([P, P], f16, tag="ptr")
                    nc.tensor.transpose(out=pt3[:, 0:8],
                                        in_=hr[:, c * P : (c + 1) * P],
                                        identity=ident[0:8, 0:8])
                    nc.vector.tensor_copy(out=hT[:, c, :], in_=pt3[:, 0:8])
                pmf = psm2.tile([8, DOUT], f32, tag="pm2")
                for c in range(DH // P):
                    nc.tensor.matmul(out=pmf[:], lhsT=hT[:, c, :],
                                     rhs=Wlsb[:, c, :],
                                     start=(c == 0), stop=(c == DH // P - 1))
                mfn2 = wt.tile([8, 1], f32, tag="mfn2")
                msf = wt.tile([8, DOUT], f32, tag="msf")
                nc.scalar.activation(msf[:], pmf[:], AF.Square,
                                     accum_out=mfn2[:])
                xnf2 = wt.tile([8, 1], f32, tag="xnf2")
                nc.vector.tensor_copy(out=xnf2[:], in_=hr[:, DH : DH + 1])
                sf = _poly_horner(nc, wt, xnf2, A_COEF, 1, "fna", p=8)
                ssf = wt.tile([8, 1], f32, tag="fn_ssq")
                nc.vector.tensor_tensor(out=ssf[:], in0=sf[:], in1=sf[:],
                                        op=OP.mult)
                rf2 = wt.tile([8, 1], f32, tag="fn_r2")
                nc.vector.tensor_tensor(out=rf2[:], in0=mfn2[:], in1=ssf[:],
                                        op=OP.mult)
                ptf = _poly_horner(nc, wt, rf2, T_COEF, 1, "fnt", p=8)
                cof = wt.tile([8, 1], f32, tag="fn_co")
                nc.vector.tensor_tensor(out=cof[:], in0=sf[:], in1=ptf[:],
                                        op=OP.mult)
                Vf = wt.tile([8, DOUT], f32, tag="Vf")
                nc.vector.tensor_scalar(out=Vf[:], in0=pmf[:],
                                        scalar1=cof[:], scalar2=None,
                                        op0=OP.mult)
                x2f = wt.tile([8, 1], f32, tag="x2f")  # ||Vf||^2
                nc.vector.tensor_tensor(out=x2f[:], in0=cof[:], in1=cof[:],
                                        op=OP.mult)
                nc.vector.tensor_tensor(out=x2f[:], in0=x2f[:], in1=mfn2[:],
                                        op=OP.mult)
                dotf = wt.tile([8, 1], f32, tag="dotf")
                prodf = wt.tile([8, DOUT], f32, tag="prodf")
                nc.vector.scalar_tensor_tensor(
                    out=prodf[:], in0=Vf[:], scalar=1.0, in1=blb[:],
                    op0=OP.mult, op1=OP.mult, accum_out=dotf[:])
                u1f = wt.tile([8, 1], f32, tag="u1f")  # 2d + bln2
                nc.vector.scalar_tensor_tensor(out=u1f[:], in0=dotf[:],
                                               scalar=2.0, in1=bln2[:],
                                               op0=OP.mult, op1=OP.add)
                t2f = wt.tile([8, 1], f32, tag="t2f")
                nc.vector.tensor_tensor(out=t2f[:], in0=x2f[:], in1=bln2[:],
                                        op=OP.mult)
                denf = wt.tile([8, 1], f32, tag="denf")
                nc.vector.scalar_tensor_tensor(out=denf[:], in0=t2f[:],
                                               scalar=1.0, in1=u1f[:],
                                               op0=OP.add, op1=OP.add)
                rdf = wt.tile([8, 1], f32, tag="rdf")
                nc.vector.reciprocal(rdf[:], denf[:])
                caf = wt.tile([8, 1], f32, tag="caf")
                nc.vector.tensor_scalar(out=caf[:], in0=u1f[:], scalar1=1.0,
                                        scalar2=None, op0=OP.add)
                s1f = wt.tile([8, 1], f32, tag="s1f")
                nc.vector.tensor_tensor(out=s1f[:], in0=caf[:], in1=rdf[:],
                                        op=OP.mult)
                wf = wt.tile([8, 1], f32, tag="wf")
                nc.vector.tensor_scalar(out=wf[:], in0=x2f[:], scalar1=-1.0,
                                        scalar2=1.0, op0=OP.mult, op1=OP.add)
                s2f = wt.tile([8, 1], f32, tag="s2f")
                nc.vector.tensor_tensor(out=s2f[:], in0=wf[:], in1=rdf[:],
                                        op=OP.mult)
                vs1 = wt.tile([8, DOUT], f32, tag="vs1")
                nc.vector.tensor_scalar(out=vs1[:], in0=Vf[:],
                                        scalar1=s1f[:], scalar2=None,
                                        op0=OP.mult)
                outt = wt.tile([8, DOUT], f32, tag="outt")
                nc.vector.scalar_tensor_tensor(
                    out=outt[:], in0=blb[:], scalar=s2f[:], in1=vs1[:],
                    op0=OP.mult, op1=OP.add)
                nc.sync.dma_start(out[:], outt[:])
            else:
                dumt = wt.tile([8, DOUT], f32, tag="dumt")
                nc.vector.memset(dumt[:], 0.0)
                nc.sync.dma_start(out[:], dumt[:])
            if KDEBUG:
                nc.sync.dma_start(dbg_t1[:], t1sh[:])
                if STAGE >= 3:
                    nc.sync.dma_start(dbg_h1[:], h1sh[:])
                if STAGE >= 4:
                    nc.sync.dma_start(dbg_h2[:], h2sh[:])

    nc.compile()
    return nc


def _get_nc():
    if "nc" not in _NC_CACHE:
        _NC_CACHE["nc"] = _build_nc()
    return _NC_CACHE["nc"]


def _prep_indices(src_idx):
    """Remap global node ids to chunk-gathered table rows, bias to int16,
    arrange per (core, group, k, queue) in dma_gather wrapped layout."""
    # unchunked AllGather: full-table row for node n is just n
    remap = np.arange(N, dtype=np.int64)

    idx_all = np.zeros((NCORES, P, NG * KDEG * NCALL * IBS), dtype=np.int16)
    for cre in range(NCORES):
        si = src_idx[cre * NSH : (cre + 1) * NSH]  # [NSH, K]
        r = remap[si] - 32768  # [NSH, K] int64 in [-32768, 32767]
        cols = []
        for gi in range(NG):
            for k in range(KDEG):
                for j in range(NCALL):
                    lo = gi * G * P + j * TPQ * P
                    lst = r[lo : lo + TPQ * P, k]  # order: idx i = blk*128+p
                    lst = np.concatenate(
                        [lst, np.zeros(NI - TPQ * P, dtype=np.int64)])
                    blk = lst.reshape(IB, 16).T.astype(np.int16)  # [16, IB]
                    blk = np.pad(blk, ((0, 0), (0, IBS - IB)))
                    cols.append(np.tile(blk, (8, 1)))  # [128, IBS]
        idx_all[cre] = np.concatenate(cols, axis=1)
    return idx_all


def kernel(features, W1, b1, W2, b2, Wl, bl, src_idx, to_fetch):
    global LAST_EXEC_NS, LAST_RESULT
    nc = _get_nc()
    features = np.asarray(features, dtype=np.float32)
    src_idx = np.asarray(src_idx, dtype=np.int64)
    to_fetch = np.asarray(to_fetch, dtype=np.int64)
    import ml_dtypes
    W1b = np.ascontiguousarray(np.asarray(W1)).astype(ml_dtypes.bfloat16)
    b1h = np.asarray(b1, np.float16).reshape(1, DH)
    W2h = np.ascontiguousarray(np.asarray(W2)).astype(np.float16)
    b2h = np.asarray(b2, np.float16).reshape(1, DH)
    WlTh = np.ascontiguousarray(np.asarray(Wl).T).astype(np.float16)
    blf = np.asarray(bl, np.float32).reshape(1, DOUT)

    idx_all = _prep_indices(src_idx)

    in_maps = []
    for c in range(NCORES):
        fsh = features[c * NSH : (c + 1) * NSH]
        featR = np.ascontiguousarray(fsh).astype(ml_dtypes.bfloat16)
        featT = np.ascontiguousarray(fsh.T).astype(ml_dtypes.bfloat16)
        bidx = np.arange(c * 8, (c + 1) * 8, dtype=np.int64)
        selv = (to_fetch[bidx] + bidx * (N // 64) - c * NSH).astype(
            np.int32).reshape(8, 1)
        in_maps.append({
            "featT": featT, "featR": featR, "idx": idx_all[c], "sel": selv,
            "W1": W1b, "b1": b1h, "W2": W2h, "b2": b2h, "WlT": WlTh,
            "bl": blf,
        })
    res = run_bass_kernel_spmd(nc, in_maps, core_ids=list(range(NCORES)),
                               trace=TRACE)
    LAST_RESULT = res
    LAST_EXEC_NS = res.exec_time_ns
    return np.concatenate([res.results[c]["out"] for c in range(NCORES)],
                          axis=0)


# revision 17
# speedup vs baseline: 1.1012x; 1.1012x over previous
"""Hyperbolic GNN classifier on 8 Trainium2 NeuronCores (Bass/Tile), v2.

Node-sharded (8192 nodes/core). Per layer: local transform -> fp16 table
(rows: 256 feats + ||row||^2 at col 256, padded to 384 elems = 768B) ->
chunked AllGather -> 16-step Mobius left-fold using batched dma_gather
(int16 indices biased by -32768 against a base offset +32768 rows; 4
SWDGE queues emit descriptors on 4 Q7 core pairs in parallel).

Fold state per row: stored direction A (fp16) with scalars (alpha, nA2):
point x = alpha*A, nA2 = ||A||^2. Per step: per-tile STT dot accumulate
d = <A,y>, fp32 scalar chain on [128, G] tiles, per-tile STT axpy
A += beta*y. All transcendentals are even-function polynomials in
squared norms (atanh(u^.5)/u^.5, tanh(v^.5)/v^.5) evaluated on DVE --
the scalar engine only ever runs Square (one ACT table set, no reloads).
"""

import os

import numpy as np

import concourse.bass as bass
import concourse.bacc as bacc
import concourse.mybir as mybir
import concourse.tile as tile
from concourse.bass_utils import run_bass_kernel_spmd
from concourse.masks import make_identity
from concourse.tile_rust import add_dep_helper

f32 = mybir.dt.float32
f16 = mybir.dt.float16
bf16 = mybir.dt.bfloat16
i32 = mybir.dt.int32
i16 = mybir.dt.int16
OP = mybir.AluOpType
AF = mybir.ActivationFunctionType

NCORES = 8
N = 65536
KDEG = 16
DIN = 512
DH = 256
DOUT = 64
P = 128
NSH = N // NCORES            # 8192 nodes per core
TILES = NSH // P             # 64 tiles per core
SE = 384                     # table row elems fp16 (768B); col 256 = y2
G = 32                       # tiles per fold group
NG = TILES // G              # 2 groups
NQ = 4                       # SWDGE queues
NCALL = 4                    # gather calls per group-step
TPQ = G // NCALL             # tiles per gather call (8)
NI = TPQ * P + 16            # idxs per call incl 16-pad (1040)
IB = (NI + 15) // 16         # idx cols actually read per call (65)
IBS = 80                     # idx block stride in cols (160B, 32B-aligned)
NCH = 4                      # collective chunks per layer (16 tiles each)
CHROWS = NSH // NCH          # 2048 shard rows per chunk
TG = 4                       # transform tiles per chain batch
NORM = float(KDEG) ** -0.5   # 0.25
NORM2 = 1.0 / KDEG

STAGE = int(os.environ.get("KSTAGE", "5"))
KDEBUG = bool(int(os.environ.get("KDEBUG", "0")))
TRACE = False
LAST_RESULT = None
LAST_EXEC_NS = None
_NC_CACHE = {}


def _poly_horner(nc, pool, u, coeffs, g, tag, p=P):
    """p(u) = 1 + u*(c1 + u*(c2 + ... )) on [p, g] fp32 DVE tiles.

    coeffs = [c1, c2, ...] low-order first. Returns the poly tile.
    """
    V = nc.vector
    cs = coeffs[::-1]  # high order first
    h = pool.tile([p, g], f32, tag=f"{tag}_h")
    if len(cs) == 1:
        V.tensor_scalar(out=h[:], in0=u[:], scalar1=cs[0], scalar2=1.0,
                        op0=OP.mult, op1=OP.add)
        return h
    V.tensor_scalar(out=h[:], in0=u[:], scalar1=cs[0], scalar2=cs[1],
                    op0=OP.mult, op1=OP.add)
    for c in cs[2:]:
        t = pool.tile([p, g], f32, tag=f"{tag}_t")
        V.tensor_tensor(out=t[:], in0=h[:], in1=u[:], op=OP.mult)
        h = pool.tile([p, g], f32, tag=f"{tag}_h")
        V.tensor_scalar(out=h[:], in0=t[:], scalar1=1.0, scalar2=c,
                        op0=OP.mult, op1=OP.add)
    t = pool.tile([p, g], f32, tag=f"{tag}_t")
    V.tensor_tensor(out=t[:], in0=h[:], in1=u[:], op=OP.mult)
    h = pool.tile([p, g], f32, tag=f"{tag}_hf")
    V.tensor_scalar(out=h[:], in0=t[:], scalar1=1.0, scalar2=1.0,
                    op0=OP.mult, op1=OP.add)
    return h


# atanh(sqrt(u))/sqrt(u) = 1 + u/3 + u^2/5 + u^3/7 + u^4/9
A_COEF = [1.0 / 3, 1.0 / 5, 1.0 / 7, 1.0 / 9]
# tanh(sqrt(v))/sqrt(v) = 1 - v/3 + 2v^2/15 - 17v^3/315
T_COEF = [-1.0 / 3, 2.0 / 15, -17.0 / 315]


def _scaled(coeffs, s):
    return [c * (s ** (i + 1)) for i, c in enumerate(coeffs)]


def _fold_chain(nc, ch, S2, y2, alpha, nA2, g):
    """Mobius fold scalars: given S2=2<A,y>, y2, state (alpha,nA2) ->
    (beta, alpha', nA2'). All [P, g] fp32."""
    V = nc.vector
    t1 = ch.tile([P, g], f32, tag="fc_t1")  # 2 a d
    V.tensor_tensor(out=t1[:], in0=alpha[:], in1=S2[:], op=OP.mult)
    u1 = ch.tile([P, g], f32, tag="fc_u1")  # 2ad + y2
    V.tensor_tensor(out=u1[:], in0=t1[:], in1=y2[:], op=OP.add)
    aa = ch.tile([P, g], f32, tag="fc_aa")
    V.tensor_tensor(out=aa[:], in0=alpha[:], in1=alpha[:], op=OP.mult)
    a2 = ch.tile([P, g], f32, tag="fc_a2")
    V.tensor_tensor(out=a2[:], in0=aa[:], in1=nA2[:], op=OP.mult)
    t2 = ch.tile([P, g], f32, tag="fc_t2")
    V.tensor_tensor(out=t2[:], in0=a2[:], in1=y2[:], op=OP.mult)
    u2 = ch.tile([P, g], f32, tag="fc_u2")  # 2ad + a2*y2
    V.tensor_tensor(out=u2[:], in0=t1[:], in1=t2[:], op=OP.add)
    den = ch.tile([P, g], f32, tag="fc_den")  # 1 + 2ad + a2*y2
    V.tensor_scalar(out=den[:], in0=u2[:], scalar1=1.0, scalar2=None,
                    op0=OP.add)
    ca = ch.tile([P, g], f32, tag="fc_ca")   # 1 + 2ad + y2
    V.tensor_scalar(out=ca[:], in0=u1[:], scalar1=1.0, scalar2=None,
                    op0=OP.add)
    r = ch.tile([P, g], f32, tag="fc_r")
    V.reciprocal(r[:], den[:])
    t3 = ch.tile([P, g], f32, tag="fc_t3")   # alpha*ca
    V.tensor_tensor(out=t3[:], in0=alpha[:], in1=ca[:], op=OP.mult)
    alpha_n = ch.tile([P, g], f32, tag="fc_an")
    V.tensor_tensor(out=alpha_n[:], in0=t3[:], in1=r[:], op=OP.mult)
    w = ch.tile([P, g], f32, tag="fc_w")     # 1 - a2
    V.tensor_scalar(out=w[:], in0=a2[:], scalar1=-1.0, scalar2=1.0,
                    op0=OP.mult, op1=OP.add)
    vinv = ch.tile([P, g], f32, tag="fc_vi")
    V.reciprocal(vinv[:], t3[:])
    beta = ch.tile([P, g], f32, tag="fc_be")
    V.tensor_tensor(out=beta[:], in0=w[:], in1=vinv[:], op=OP.mult)
    t4 = ch.tile([P, g], f32, tag="fc_t4")  # beta * 2d
    V.tensor_tensor(out=t4[:], in0=beta[:], in1=S2[:], op=OP.mult)
    t5 = ch.tile([P, g], f32, tag="fc_t5")   # nA2 + 2 beta d
    V.tensor_tensor(out=t5[:], in0=t4[:], in1=nA2[:], op=OP.add)
    t6 = ch.tile([P, g], f32, tag="fc_t6")
    V.tensor_tensor(out=t6[:], in0=beta[:], in1=beta[:], op=OP.mult)
    t7 = ch.tile([P, g], f32, tag="fc_t7")
    V.tensor_tensor(out=t7[:], in0=t6[:], in1=y2[:], op=OP.mult)
    nA2_n = ch.tile([P, g], f32, tag="fc_nn")
    V.tensor_tensor(out=nA2_n[:], in0=t5[:], in1=t7[:], op=OP.add)
    return beta, alpha_n, nA2_n


def _acc_row(acc, t):
    return acc[:, t // TPQ, t % TPQ, 0:DH]


def _build_nc():
    nc = bacc.Bacc("TRN2", target_bir_lowering=False, debug=False,
                   num_devices=NCORES, num_swdge_queues=NQ)
    featT = nc.dram_tensor("featT", [DIN, NSH], bf16, kind="ExternalInput")
    featR = nc.dram_tensor("featR", [NSH, DIN], bf16, kind="ExternalInput")
    idx = nc.dram_tensor("idx", [P, NG * KDEG * NCALL * IBS], i16,
                         kind="ExternalInput")
    sel = nc.dram_tensor("sel", [8, 1], i32, kind="ExternalInput")
    W1 = nc.dram_tensor("W1", [DIN, DH], bf16, kind="ExternalInput")
    b1 = nc.dram_tensor("b1", [1, DH], f16, kind="ExternalInput")
    W2 = nc.dram_tensor("W2", [DH, DH], f16, kind="ExternalInput")
    b2 = nc.dram_tensor("b2", [1, DH], f16, kind="ExternalInput")
    WlT = nc.dram_tensor("WlT", [DH, DOUT], f16, kind="ExternalInput")
    bl = nc.dram_tensor("bl", [1, DOUT], f32, kind="ExternalInput")
    out = nc.dram_tensor("out", [8, DOUT], f32, kind="ExternalOutput")
    if KDEBUG:
        dbg_t1 = nc.dram_tensor("dbg_t1", [NSH, 264], f16,
                                kind="ExternalOutput")
        dbg_h1 = nc.dram_tensor("dbg_h1", [NSH, 264], f16,
                                kind="ExternalOutput")
        dbg_h2 = nc.dram_tensor("dbg_h2", [NSH, 264], f16,
                                kind="ExternalOutput")

    with tile.TileContext(nc) as tc:
        with (
            tc.tile_pool(name="wt", bufs=1) as wt,
            tc.tile_pool(name="sb", bufs=3) as sb,
            tc.tile_pool(name="ch", bufs=2) as ch,
            tc.tile_pool(name="yb", bufs=2) as yb,
            tc.tile_pool(name="ac", bufs=1) as ac,
            tc.tile_pool(name="df", bufs=2) as df,
            tc.tile_pool(name="tp", bufs=1) as tp,
            tc.tile_pool(name="ps", bufs=2, space="PSUM") as ps,
            tc.tile_pool(name="psmx", bufs=4, space="PSUM") as psmx,
            tc.tile_pool(name="psm2", bufs=2, space="PSUM") as psm2,
            tc.tile_pool(name="dr", bufs=1, space="DRAM") as dr,
        ):
            ident = wt.tile([P, P], f16, tag="ident")
            make_identity(nc, ident[:])

            # ---- weights ----
            W1sb = wt.tile([P, DIN // P, DH], bf16, tag="W1sb")
            nc.sync.dma_start(W1sb[:],
                              W1[:].rearrange("(a p) d -> p a d", p=P))
            W2sb = wt.tile([P, DH // P, DH], f16, tag="W2sb")
            nc.sync.dma_start(W2sb[:],
                              W2[:].rearrange("(a p) d -> p a d", p=P))
            Wlsb = wt.tile([P, DH // P, DOUT], f16, tag="Wlsb")
            nc.sync.dma_start(Wlsb[:],
                              WlT[:].rearrange("(a p) d -> p a d", p=P))
            b1row = wt.tile([1, DH], f16, tag="b1row")
            nc.sync.dma_start(b1row[:], b1[:])
            b1b = wt.tile([P, DH], f16, tag="b1b")
            nc.gpsimd.partition_broadcast(b1b[:], b1row[:])
            b2row = wt.tile([1, DH], f16, tag="b2row")
            nc.sync.dma_start(b2row[:], b2[:])
            b2b = wt.tile([P, DH], f16, tag="b2b")
            nc.gpsimd.partition_broadcast(b2b[:], b2row[:])
            blrow = wt.tile([1, DOUT], f32, tag="blrow")
            nc.sync.dma_start(blrow[:], bl[:])
            blb = wt.tile([8, DOUT], f32, tag="blb")
            nc.gpsimd.partition_broadcast(blb[:], blrow[:], channels=8)
            bscr = wt.tile([P, DH], f16, tag="bscr")
            b1n2 = wt.tile([P, 1], f32, tag="b1n2")
            nc.scalar.activation(bscr[:], b1b[:], AF.Square, accum_out=b1n2[:])
            bscr2 = wt.tile([P, DH], f16, tag="bscr2")
            b2n2 = wt.tile([P, 1], f32, tag="b2n2")
            nc.scalar.activation(bscr2[:], b2b[:], AF.Square,
                                 accum_out=b2n2[:])
            bscr3 = wt.tile([8, DOUT], f32, tag="bscr3")
            bln2 = wt.tile([8, 1], f32, tag="bln2")
            nc.scalar.activation(bscr3[:], blb[:], AF.Square,
                                 accum_out=bln2[:])

            # ---- indices (biased int16, replicated over partitions) ----
            idx_sb = wt.tile([P, NG * KDEG * NCALL * IBS], i16, tag="idx")
            nc.sync.dma_start(idx_sb[:], idx[:])

            def idx_slice(gi, k, j):
                o = (((gi * KDEG) + k) * NCALL + j) * IBS
                return idx_sb[:, o : o + IB]

            # ---- DRAM tables ----
            t1sh = dr.tile([NSH, SE], f16, tag="t1sh")
            t1full = dr.tile([N, SE], f16, tag="t1full", addr_space="Shared")
            h1sh = dr.tile([NSH, SE], f16, tag="h1sh")
            h1full = dr.tile([N, SE], f16, tag="h1full", addr_space="Shared")
            h2sh = dr.tile([NSH, SE], f16, tag="h2sh")

            # ============ Phase T: layer-1 transform ============
            cc_insts_1 = []
            for bt in (range(0, TILES, TG) if STAGE >= 1 else []):
                xn2 = ch.tile([P, TG], f32, tag="tf_xn2")
                mxn2 = ch.tile([P, TG], f32, tag="tf_mxn2")
                pmx_list = []
                for j in range(TG):
                    t = bt + j
                    ftT = sb.tile([P, DIN // P, P], bf16, tag="ftT")
                    nc.sync.dma_start(
                        ftT[:],
                        featT[:, t * P : (t + 1) * P].rearrange(
                            "(c p) n -> p c n", p=P))
                    ftR = sb.tile([P, DIN], bf16, tag="ftR")
                    nc.sync.dma_start(ftR[:], featR[t * P : (t + 1) * P, :])
                    sq = sb.tile([P, DIN], bf16, tag="sq")
                    nc.scalar.activation(sq[:], ftR[:], AF.Square,
                                         accum_out=xn2[:, j : j + 1])
                    pmx = psmx.tile([P, DH], f32, tag="pmx")
                    for c in range(DIN // P):
                        nc.tensor.matmul(out=pmx[:], lhsT=ftT[:, c, :],
                                         rhs=W1sb[:, c, :],
                                         start=(c == 0),
                                         stop=(c == DIN // P - 1))
                    msq = sb.tile([P, DH], f16, tag="msq")
                    nc.scalar.activation(msq[:], pmx[:], AF.Square,
                                         accum_out=mxn2[:, j : j + 1])
                    pmx_list.append(pmx)
                # batched scalar chain: s=poly_a(xn2/16), r2=(mxn2/16)*s^2
                s = _poly_horner(nc, ch, xn2, _scaled(A_COEF, NORM2), TG,
                                 "tfa")
                ssq = ch.tile([P, TG], f32, tag="tf_ssq")
                nc.vector.tensor_tensor(out=ssq[:], in0=s[:], in1=s[:],
                                        op=OP.mult)
                mxs = ch.tile([P, TG], f32, tag="tf_mxs")
                nc.vector.tensor_tensor(out=mxs[:], in0=mxn2[:], in1=ssq[:],
                                        op=OP.mult)
                pt = _poly_horner(nc, ch, mxs, _scaled(T_COEF, NORM2), TG,
                                  "tft")
                spt = ch.tile([P, TG], f32, tag="tf_spt")
                nc.vector.tensor_tensor(out=spt[:], in0=s[:], in1=pt[:],
                                        op=OP.mult)
                coef = ch.tile([P, TG], f32, tag="tf_coef")
                nc.vector.tensor_scalar(out=coef[:], in0=spt[:],
                                        scalar1=NORM, scalar2=None,
                                        op0=OP.mult)
                # y2 = (r*pt)^2 = (mxs/16)*pt^2 * s^2... careful:
                # row = coef*pmx; y2 = coef^2 * mxn2
                c2 = ch.tile([P, TG], f32, tag="tf_c2")
                nc.vector.tensor_tensor(out=c2[:], in0=coef[:], in1=coef[:],
                                        op=OP.mult)
                y2r = ch.tile([P, TG], f32, tag="tf_y2r")
                nc.vector.tensor_tensor(out=y2r[:], in0=c2[:], in1=mxn2[:],
                                        op=OP.mult)
                for j in range(TG):
                    t = bt + j
                    htile = sb.tile([P, 257], f16, tag="htile")
                    nc.vector.tensor_scalar(
                        out=htile[:, 0:DH], in0=pmx_list[j][:],
                        scalar1=coef[:, j : j + 1], scalar2=None, op0=OP.mult)
                    nc.vector.tensor_copy(out=htile[:, DH : DH + 1],
                                          in_=y2r[:, j : j + 1])
                    nc.sync.dma_start(t1sh[t * P : (t + 1) * P, 0:257],
                                      htile[:])

            if STAGE >= 2:
                cc = nc.gpsimd.collective_compute(
                    "AllGather", OP.bypass,
                    replica_groups=[list(range(NCORES))],
                    ins=[t1sh[:]], outs=[t1full[:]])
                cc_insts_1.append(cc)

            # ============ fold machinery ============
            def fold_layer(table, cc_insts, bias_b, bias_n2, matvec2,
                           dst_sh):
                """One aggregation layer over `table` (gather source).

                matvec2: if True apply W2 mobius matvec after the fold
                (layer 2); else layer-1 style (store with NORM scale).
                """
                base = table[N // 2 :, :]
                for gi in range(NG):
                    acc = ac.tile([P, NCALL, TPQ + 1, SE], f16, tag="acc")
                    gaths = []
                    for j in range(NCALL):
                        g0 = nc.gpsimd.dma_gather(
                            out_ap=acc[:, j, :, :], in_ap=base,
                            idxs_ap=idx_slice(gi, 0, j),
                            num_idxs=NI, num_idxs_reg=NI, elem_size=SE,
                            single_packet=False, queue_num=j % NQ)
                        gaths.append(g0)
                    alpha = ch.tile([P, G], f32, tag="st_alpha")
                    nc.vector.memset(alpha[:], 1.0)
                    nA2 = ch.tile([P, G], f32, tag="st_nA2")
                    nc.vector.tensor_copy(
                        out=nA2[:].rearrange("p (q b) -> p q b", q=NCALL),
                        in_=acc[:, :, 0:TPQ, DH])
                    scratch = sb.tile([P, DH], f16, tag="dotscr")

                    for k in range(1, KDEG):
                        ybuf = yb.tile([P, NCALL, TPQ + 1, SE], f16,
                                       tag="ybuf")
                        for j in range(NCALL):
                            gk = nc.gpsimd.dma_gather(
                                out_ap=ybuf[:, j, :, :], in_ap=base,
                                idxs_ap=idx_slice(gi, k, j),
                                num_idxs=NI, num_idxs_reg=NI, elem_size=SE,
                                single_packet=False, queue_num=j % NQ)
                            gaths.append(gk)
                        prod = df.tile([P, NCALL, TPQ, DH], bf16, tag="prod")
                        nc.vector.tensor_tensor(
                            out=prod[:], in0=acc[:, :, 0:TPQ, 0:DH],
                            in1=ybuf[:, :, 0:TPQ, 0:DH], op=OP.mult)
                        red = df.tile([P, NCALL, TPQ, DH // 2], bf16,
                                      tag="red")
                        nc.vector.tensor_tensor(
                            out=red[:], in0=prod[:, :, :, 0 : DH // 2],
                            in1=prod[:, :, :, DH // 2 : DH], op=OP.add)
                        w = DH // 2
                        while w > 4:
                            nc.vector.tensor_tensor(
                                out=red[:, :, :, 0 : w // 2],
                                in0=red[:, :, :, 0 : w // 2],
                                in1=red[:, :, :, w // 2 : w], op=OP.add)
                            w //= 2
                        d = ch.tile([P, G], f32, tag="fd_d")
                        nc.vector.tensor_reduce(
                            out=d[:].rearrange("p (q b) -> p q b", q=NCALL),
                            in_=red[:, :, :, 0:4], axis=mybir.AxisListType.X,
                            op=OP.add)
                        y2 = ch.tile([P, G], f32, tag="fd_y2")
                        nc.vector.tensor_copy(
                            out=y2[:].rearrange("p (q b) -> p q b", q=NCALL),
                            in_=ybuf[:, :, 0:TPQ, DH])
                        S2 = ch.tile([P, G], f32, tag="fd_S2")
                        nc.vector.tensor_scalar(out=S2[:], in0=d[:],
                                                scalar1=2.0, scalar2=None,
                                                op0=OP.mult)
                        beta, alpha, nA2 = _fold_chain(nc, ch, S2, y2,
                                                       alpha, nA2, G)
                        b16 = ch.tile([P, G], f16, tag="fd_b16")
                        nc.vector.tensor_copy(out=b16[:], in_=beta[:])
                        bbc = b16[:].rearrange(
                            "p (q b) -> p q b", q=NCALL).unsqueeze(
                            3).to_broadcast([P, NCALL, TPQ, DH])
                        tmp = tp.tile([P, NCALL, TPQ, DH], f16, tag="tmp")
                        nc.vector.tensor_tensor(
                            out=tmp[:], in0=ybuf[:, :, 0:TPQ, 0:DH],
                            in1=bbc, op=OP.mult)
                        nc.vector.tensor_tensor(
                            out=acc[:, :, 0:TPQ, 0:DH],
                            in0=acc[:, :, 0:TPQ, 0:DH], in1=tmp[:],
                            op=OP.add)
                    for gth in gaths:
                        for cci in cc_insts:
                            add_dep_helper(gth.ins, cci.ins,
                                           reason="gather after allgather")

                    if matvec2:
                        # x = alpha*A; mx = alpha*(A @ W2); v-rows = pmA
                        a2 = ch.tile([P, G], f32, tag="mv_a2")
                        nc.vector.tensor_tensor(out=a2[:], in0=alpha[:],
                                                in1=alpha[:], op=OP.mult)
                        nc.vector.tensor_tensor(out=a2[:], in0=a2[:],
                                                in1=nA2[:], op=OP.mult)
                        s2g = _poly_horner(nc, ch, a2, A_COEF, G, "mva")
                        msqA = ch.tile([P, G], f32, tag="mv_msq")
                        for t in range(G):
                            aT = sb.tile([P, DH // P, P], f16, tag="aT")
                            for c in range(DH // P):
                                ptr = ps.tile([P, P], f16, tag="ptr")
                                nc.tensor.transpose(
                                    out=ptr[:],
                                    in_=acc[:, t // TPQ, t % TPQ,
                                            c * P : (c + 1) * P],
                                    identity=ident[:])
                                nc.vector.tensor_copy(out=aT[:, c, :],
                                                      in_=ptr[:])
                            pm2 = psm2.tile([P, DH], f32, tag="pm2")
                            for c in range(DH // P):
                                nc.tensor.matmul(out=pm2[:],
                                                 lhsT=aT[:, c, :],
                                                 rhs=W2sb[:, c, :],
                                                 start=(c == 0),
                                                 stop=(c == DH // P - 1))
                            ms2 = sb.tile([P, DH], f16, tag="ms2")
                            nc.scalar.activation(ms2[:], pm2[:], AF.Square,
                                                 accum_out=msqA[:, t : t + 1])
                            # overwrite acc row with pmA (fp16)
                            nc.vector.tensor_copy(
                                out=acc[:, t // TPQ, t % TPQ, 0:DH],
                                in_=pm2[:])
                        # r2 = a2 * msqA * s^2; coef = alpha*s*pt(r2)*NORM
                        ssq = ch.tile([P, G], f32, tag="mv_ssq")
                        nc.vector.tensor_tensor(out=ssq[:], in0=s2g[:],
                                                in1=s2g[:], op=OP.mult)
                        aa2 = ch.tile([P, G], f32, tag="mv_aa2")
                        nc.vector.tensor_tensor(out=aa2[:], in0=alpha[:],
                                                in1=alpha[:], op=OP.mult)
                        r2 = ch.tile([P, G], f32, tag="mv_r2")
                        nc.vector.tensor_tensor(out=r2[:], in0=aa2[:],
                                                in1=msqA[:], op=OP.mult)
                        nc.vector.tensor_tensor(out=r2[:], in0=r2[:],
                                                in1=ssq[:], op=OP.mult)
                        ptg = _poly_horner(nc, ch, r2, T_COEF, G, "mvt")
                        co = ch.tile([P, G], f32, tag="mv_co")
                        nc.vector.tensor_tensor(out=co[:], in0=alpha[:],
                                                in1=s2g[:], op=OP.mult)
                        nc.vector.tensor_tensor(out=co[:], in0=co[:],
                                                in1=ptg[:], op=OP.mult)
                        alpha = ch.tile([P, G], f32, tag="st_alpha")
                        nc.vector.tensor_scalar(out=alpha[:], in0=co[:],
                                                scalar1=NORM, scalar2=None,
                                                op0=OP.mult)
                        nA2 = msqA
                        bias_row, bias_sq = b2b, b2n2
                        scale = 1.0
                    else:
                        alpha_s = ch.tile([P, G], f32, tag="st_alpha")
                        nc.vector.tensor_scalar(out=alpha_s[:], in0=alpha[:],
                                                scalar1=NORM, scalar2=None,
                                                op0=OP.mult)
                        alpha = alpha_s
                        bias_row, bias_sq = bias_b, bias_n2
                        scale = NORM

                    # bias fold: y = bias_row (broadcast), y2 = bias_sq
                    d = ch.tile([P, G], f32, tag="fd_d")
                    for t in range(G):
                        nc.vector.scalar_tensor_tensor(
                            out=scratch[:], in0=_acc_row(acc, t), scalar=1.0,
                            in1=bias_row[:], op0=OP.mult, op1=OP.mult,
                            accum_out=d[:, t : t + 1])
                    S2b = ch.tile([P, G], f32, tag="fd_S2b")
                    nc.vector.tensor_scalar(out=S2b[:], in0=d[:], scalar1=2.0,
                                            scalar2=None, op0=OP.mult)
                    y2b = ch.tile([P, G], f32, tag="fd_y2b")
                    nc.vector.tensor_scalar(out=y2b[:], in0=d[:], scalar1=0.0,
                                            scalar2=bias_sq[:, 0:1],
                                            op0=OP.mult, op1=OP.add)
                    beta, alpha, nA2 = _fold_chain(nc, ch, S2b, y2b, alpha,
                                                   nA2, G)
                    for t in range(G):
                        nc.vector.scalar_tensor_tensor(
                            out=_acc_row(acc, t), in0=bias_row[:],
                            scalar=beta[:, t : t + 1], in1=_acc_row(acc, t),
                            op0=OP.mult, op1=OP.add)

                    # activation: logmap0 -> relu -> expmap0, store
                    rn2g = ch.tile([P, G], f32, tag="as_rn2")
                    for t in range(G):
                        zrow = _acc_row(acc, t)
                        nc.vector.tensor_scalar(out=zrow, in0=zrow,
                                                scalar1=0.0, scalar2=None,
                                                op0=OP.max)
                        zsq = sb.tile([P, DH], f16, tag="zsq")
                        nc.scalar.activation(zsq[:], zrow, AF.Square,
                                             accum_out=rn2g[:, t : t + 1])
                    a2f = ch.tile([P, G], f32, tag="as_a2f")
                    nc.vector.tensor_tensor(out=a2f[:], in0=alpha[:],
                                            in1=alpha[:], op=OP.mult)
                    nc.vector.tensor_tensor(out=a2f[:], in0=a2f[:],
                                            in1=nA2[:], op=OP.mult)
                    sg = _poly_horner(nc, ch, a2f, A_COEF, G, "asa")
                    c1 = ch.tile([P, G], f32, tag="as_c1")
                    nc.vector.tensor_tensor(out=c1[:], in0=sg[:],
                                            in1=alpha[:], op=OP.mult)
                    c1sq = ch.tile([P, G], f32, tag="as_c1sq")
                    nc.vector.tensor_tensor(out=c1sq[:], in0=c1[:],
                                            in1=c1[:], op=OP.mult)
                    vr2 = ch.tile([P, G], f32, tag="as_vr2")
                    nc.vector.tensor_tensor(out=vr2[:], in0=c1sq[:],
                                            in1=rn2g[:], op=OP.mult)
                    ptg2 = _poly_horner(nc, ch, vr2, T_COEF, G, "ast")
                    gam = ch.tile([P, G], f32, tag="as_gam")
                    nc.vector.tensor_tensor(out=gam[:], in0=c1[:],
                                            in1=ptg2[:], op=OP.mult)
                    if scale != 1.0:
                        gs = ch.tile([P, G], f32, tag="as_gs")
                        nc.vector.tensor_scalar(out=gs[:], in0=gam[:],
                                                scalar1=scale, scalar2=None,
                                                op0=OP.mult)
                        gam = gs
                    gg = ch.tile([P, G], f32, tag="as_gg")
                    nc.vector.tensor_tensor(out=gg[:], in0=gam[:],
                                            in1=gam[:], op=OP.mult)
                    y2o = ch.tile([P, G], f32, tag="as_y2o")
                    nc.vector.tensor_tensor(out=y2o[:], in0=gg[:],
                                            in1=rn2g[:], op=OP.mult)
                    for t in range(G):
                        ti = gi * G + t
                        htile = sb.tile([P, 257], f16, tag="htile")
                        nc.vector.tensor_scalar(
                            out=htile[:, 0:DH], in0=_acc_row(acc, t),
                            scalar1=gam[:, t : t + 1], scalar2=None,
                            op0=OP.mult)
                        nc.vector.tensor_copy(out=htile[:, DH : DH + 1],
                                              in_=y2o[:, t : t + 1])
                        nc.sync.dma_start(
                            dst_sh[ti * P : (ti + 1) * P, 0:257], htile[:])
                return

            # ============ Phase A1 + AllGather h1 ============
            cc_insts_2 = []
            if STAGE >= 3:
                fold_layer(t1full, cc_insts_1, b1b, b1n2, False, h1sh)
                if STAGE >= 4:
                    cc = nc.gpsimd.collective_compute(
                        "AllGather", OP.bypass,
                        replica_groups=[list(range(NCORES))],
                        ins=[h1sh[:]], outs=[h1full[:]])
                    cc_insts_2.append(cc)

            # ============ Phase A2 ============
            if STAGE >= 4:
                fold_layer(h1full, cc_insts_2, b2b, b2n2, True, h2sh)

            # ============ Phase D: classifier ============
            if STAGE >= 5:
                selt = wt.tile([8, 1], i32, tag="selt")
                nc.sync.dma_start(selt[:], sel[:])
                hr = wt.tile([8, SE], f16, tag="hr")
                nc.gpsimd.indirect_dma_start(
                    out=hr[:], out_offset=None, in_=h2sh[:],
                    in_offset=bass.IndirectOffsetOnAxis(ap=selt[:, 0:1],
                                                        axis=0))
                hT = wt.tile([P, DH // P, 8], f16, tag="hT")
                for c in range(DH // P):
                    pt3 = ps.tile([P, P], f16, tag="ptr")
                    nc.tensor.transpose(out=pt3[:, 0:8],
                                        in_=hr[:, c * P : (c + 1) * P],
                                        identity=ident[0:8, 0:8])
                    nc.vector.tensor_copy(out=hT[:, c, :], in_=pt3[:, 0:8])
                pmf = psm2.tile([8, DOUT], f32, tag="pm2")
                for c in range(DH // P):
                    nc.tensor.matmul(out=pmf[:], lhsT=hT[:, c, :],
                                     rhs=Wlsb[:, c, :],
                                     start=(c == 0), stop=(c == DH // P - 1))
                mfn2 = wt.tile([8, 1], f32, tag="mfn2")
                msf = wt.tile([8, DOUT], f32, tag="msf")
                nc.scalar.activation(msf[:], pmf[:], AF.Square,
                                     accum_out=mfn2[:])
                xnf2 = wt.tile([8, 1], f32, tag="xnf2")
                nc.vector.tensor_copy(out=xnf2[:], in_=hr[:, DH : DH + 1])
                sf = _poly_horner(nc, wt, xnf2, A_COEF, 1, "fna", p=8)
                ssf = wt.tile([8, 1], f32, tag="fn_ssq")
                nc.vector.tensor_tensor(out=ssf[:], in0=sf[:], in1=sf[:],
                                        op=OP.mult)
                rf2 = wt.tile([8, 1], f32, tag="fn_r2")
                nc.vector.tensor_tensor(out=rf2[:], in0=mfn2[:], in1=ssf[:],
                                        op=OP.mult)
                ptf = _poly_horner(nc, wt, rf2, T_COEF, 1, "fnt", p=8)
                cof = wt.tile([8, 1], f32, tag="fn_co")
                nc.vector.tensor_tensor(out=cof[:], in0=sf[:], in1=ptf[:],
                                        op=OP.mult)
                Vf = wt.tile([8, DOUT], f32, tag="Vf")
                nc.vector.tensor_scalar(out=Vf[:], in0=pmf[:],
                                        scalar1=cof[:], scalar2=None,
                                        op0=OP.mult)
                x2f = wt.tile([8, 1], f32, tag="x2f")  # ||Vf||^2
                nc.vector.tensor_tensor(out=x2f[:], in0=cof[:], in1=cof[:],
                                        op=OP.mult)
                nc.vector.tensor_tensor(out=x2f[:], in0=x2f[:], in1=mfn2[:],
                                        op=OP.mult)
                dotf = wt.tile([8, 1], f32, tag="dotf")
                prodf = wt.tile([8, DOUT], f32, tag="prodf")
                nc.vector.scalar_tensor_tensor(
                    out=prodf[:], in0=Vf[:], scalar=1.0, in1=blb[:],
                    op0=OP.mult, op1=OP.mult, accum_out=dotf[:])
                u1f = wt.tile([8, 1], f32, tag="u1f")  # 2d + bln2
                nc.vector.scalar_tensor_tensor(out=u1f[:], in0=dotf[:],
                                               scalar=2.0, in1=bln2[:],
                                               op0=OP.mult, op1=OP.add)
                t2f = wt.tile([8, 1], f32, tag="t2f")
                nc.vector.tensor_tensor(out=t2f[:], in0=x2f[:], in1=bln2[:],
                                        op=OP.mult)
                denf = wt.tile([8, 1], f32, tag="denf")
                nc.vector.scalar_tensor_tensor(out=denf[:], in0=t2f[:],
                                               scalar=1.0, in1=u1f[:],
                                               op0=OP.add, op1=OP.add)
                rdf = wt.tile([8, 1], f32, tag="rdf")
                nc.vector.reciprocal(rdf[:], denf[:])
                caf = wt.tile([8, 1], f32, tag="caf")
                nc.vector.tensor_scalar(out=caf[:], in0=u1f[:], scalar1=1.0,
                                        scalar2=None, op0=OP.add)
                s1f = wt.tile([8, 1], f32, tag="s1f")
                nc.vector.tensor_tensor(out=s1f[:], in0=caf[:], in1=rdf[:],
                                        op=OP.mult)
                wf = wt.tile([8, 1], f32, tag="wf")
                nc.vector.tensor_scalar(out=wf[:], in0=x2f[:], scalar1=-1.0,
                                        scalar2=1.0, op0=OP.mult, op1=OP.add)
                s2f = wt.tile([8, 1], f32, tag="s2f")
                nc.vector.tensor_tensor(out=s2f[:], in0=wf[:], in1=rdf[:],
                                        op=OP.mult)
                vs1 = wt.tile([8, DOUT], f32, tag="vs1")
                nc.vector.tensor_scalar(out=vs1[:], in0=Vf[:],
                                        scalar1=s1f[:], scalar2=None,
                                        op0=OP.mult)
                outt = wt.tile([8, DOUT], f32, tag="outt")
                nc.vector.scalar_tensor_tensor(
                    out=outt[:], in0=blb[:], scalar=s2f[:], in1=vs1[:],
                    op0=OP.mult, op1=OP.add)
                nc.sync.dma_start(out[:], outt[:])
            else:
                dumt = wt.tile([8, DOUT], f32, tag="dumt")
                nc.vector.memset(dumt[:], 0.0)
                nc.sync.dma_start(out[:], dumt[:])
            if KDEBUG:
                nc.sync.dma_start(dbg_t1[:], t1sh[:])
                if STAGE >= 3:
                    nc.sync.dma_start(dbg_h1[:], h1sh[:])
                if STAGE >= 4:
                    nc.sync.dma_start(dbg_h2[:], h2sh[:])

    nc.compile()
    return nc


def _get_nc():
    if "nc" not in _NC_CACHE:
        _NC_CACHE["nc"] = _build_nc()
    return _NC_CACHE["nc"]


def _prep_indices(src_idx):
    """Remap global node ids to chunk-gathered table rows, bias to int16,
    arrange per (core, group, k, queue) in dma_gather wrapped layout."""
    # unchunked AllGather: full-table row for node n is just n
    remap = np.arange(N, dtype=np.int64)

    idx_all = np.zeros((NCORES, P, NG * KDEG * NCALL * IBS), dtype=np.int16)
    for cre in range(NCORES):
        si = src_idx[cre * NSH : (cre + 1) * NSH]  # [NSH, K]
        r = remap[si] - 32768  # [NSH, K] int64 in [-32768, 32767]
        cols = []
        for gi in range(NG):
            for k in range(KDEG):
                for j in range(NCALL):
                    lo = gi * G * P + j * TPQ * P
                    lst = r[lo : lo + TPQ * P, k]  # order: idx i = blk*128+p
                    lst = np.concatenate(
                        [lst, np.zeros(NI - TPQ * P, dtype=np.int64)])
                    blk = lst.reshape(IB, 16).T.astype(np.int16)  # [16, IB]
                    blk = np.pad(blk, ((0, 0), (0, IBS - IB)))
                    cols.append(np.tile(blk, (8, 1)))  # [128, IBS]
        idx_all[cre] = np.concatenate(cols, axis=1)
    return idx_all


def kernel(features, W1, b1, W2, b2, Wl, bl, src_idx, to_fetch):
    global LAST_EXEC_NS, LAST_RESULT
    nc = _get_nc()
    features = np.asarray(features, dtype=np.float32)
    src_idx = np.asarray(src_idx, dtype=np.int64)
    to_fetch = np.asarray(to_fetch, dtype=np.int64)
    import ml_dtypes
    W1b = np.ascontiguousarray(np.asarray(W1)).astype(ml_dtypes.bfloat16)
    b1h = np.asarray(b1, np.float16).reshape(1, DH)
    W2h = np.ascontiguousarray(np.asarray(W2)).astype(np.float16)
    b2h = np.asarray(b2, np.float16).reshape(1, DH)
    WlTh = np.ascontiguousarray(np.asarray(Wl).T).astype(np.float16)
    blf = np.asarray(bl, np.float32).reshape(1, DOUT)

    idx_all = _prep_indices(src_idx)

    in_maps = []
    for c in range(NCORES):
        fsh = features[c * NSH : (c + 1) * NSH]
        featR = np.ascontiguousarray(fsh).astype(ml_dtypes.bfloat16)
        featT = np.ascontiguousarray(fsh.T).astype(ml_dtypes.bfloat16)
        bidx = np.arange(c * 8, (c + 1) * 8, dtype=np.int64)
        selv = (to_fetch[bidx] + bidx * (N // 64) - c * NSH).astype(
            np.int32).reshape(8, 1)
        in_maps.append({
            "featT": featT, "featR": featR, "idx": idx_all[c], "sel": selv,
            "W1": W1b, "b1": b1h, "W2": W2h, "b2": b2h, "WlT": WlTh,
            "bl": blf,
        })
    res = run_bass_kernel_spmd(nc, in_maps, core_ids=list(range(NCORES)),
                               trace=TRACE)
    LAST_RESULT = res
    LAST_EXEC_NS = res.exec_time_ns
    return np.concatenate([res.results[c]["out"] for c in range(NCORES)],
                          axis=0)


# revision 20
# speedup vs baseline: 1.2031x; 1.0925x over previous
"""Hyperbolic GNN classifier on 8 Trainium2 NeuronCores (Bass/Tile), v2.

Node-sharded (8192 nodes/core). Per layer: local transform -> fp16 table
(rows: 256 feats + ||row||^2 at col 256, padded to 384 elems = 768B) ->
chunked AllGather -> 16-step Mobius left-fold using batched dma_gather
(int16 indices biased by -32768 against a base offset +32768 rows; 4
SWDGE queues emit descriptors on 4 Q7 core pairs in parallel).

Fold state per row: stored direction A (fp16) with scalars (alpha, nA2):
point x = alpha*A, nA2 = ||A||^2. Per step: per-tile STT dot accumulate
d = <A,y>, fp32 scalar chain on [128, G] tiles, per-tile STT axpy
A += beta*y. All transcendentals are even-function polynomials in
squared norms (atanh(u^.5)/u^.5, tanh(v^.5)/v^.5) evaluated on DVE --
the scalar engine only ever runs Square (one ACT table set, no reloads).
"""

import os

import numpy as np

import concourse.bass as bass
import concourse.bacc as bacc
import concourse.mybir as mybir
import concourse.tile as tile
from concourse.bass_utils import run_bass_kernel_spmd
from concourse.masks import make_identity
from concourse.tile_rust import add_dep_helper

f32 = mybir.dt.float32
f16 = mybir.dt.float16
bf16 = mybir.dt.bfloat16
i32 = mybir.dt.int32
i16 = mybir.dt.int16
OP = mybir.AluOpType
AF = mybir.ActivationFunctionType

NCORES = 8
N = 65536
KDEG = 16
DIN = 512
DH = 256
DOUT = 64
P = 128
NSH = N // NCORES            # 8192 nodes per core
TILES = NSH // P             # 64 tiles per core
SE = 384                     # table row elems fp16 (768B); col 256 = y2
G = 32                       # tiles per fold group
NG = TILES // G              # 2 groups
NQ = 4                       # SWDGE queues
NCALL = 4                    # gather calls per group-step
TPQ = G // NCALL             # tiles per gather call (8)
NI = TPQ * P + 16            # idxs per call incl 16-pad (1040)
IB = (NI + 15) // 16         # idx cols actually read per call (65)
IBS = 80                     # idx block stride in cols (160B, 32B-aligned)
NCH = 4                      # collective chunks per layer (16 tiles each)
CHROWS = NSH // NCH          # 2048 shard rows per chunk
TG = 4                       # transform tiles per chain batch
NORM = float(KDEG) ** -0.5   # 0.25
NORM2 = 1.0 / KDEG

STAGE = int(os.environ.get("KSTAGE", "5"))
KDEBUG = bool(int(os.environ.get("KDEBUG", "0")))
TRACE = False
LAST_RESULT = None
LAST_EXEC_NS = None
_NC_CACHE = {}


def _poly_horner(nc, pool, u, coeffs, g, tag, p=P):
    """p(u) = 1 + u*(c1 + u*(c2 + ... )) on [p, g] fp32 DVE tiles.

    coeffs = [c1, c2, ...] low-order first. Returns the poly tile.
    """
    V = nc.vector
    cs = coeffs[::-1]  # high order first
    h = pool.tile([p, g], f32, tag=f"{tag}_h")
    if len(cs) == 1:
        V.tensor_scalar(out=h[:], in0=u[:], scalar1=cs[0], scalar2=1.0,
                        op0=OP.mult, op1=OP.add)
        return h
    V.tensor_scalar(out=h[:], in0=u[:], scalar1=cs[0], scalar2=cs[1],
                    op0=OP.mult, op1=OP.add)
    for c in cs[2:]:
        t = pool.tile([p, g], f32, tag=f"{tag}_t")
        V.tensor_tensor(out=t[:], in0=h[:], in1=u[:], op=OP.mult)
        h = pool.tile([p, g], f32, tag=f"{tag}_h")
        V.tensor_scalar(out=h[:], in0=t[:], scalar1=1.0, scalar2=c,
                        op0=OP.mult, op1=OP.add)
    t = pool.tile([p, g], f32, tag=f"{tag}_t")
    V.tensor_tensor(out=t[:], in0=h[:], in1=u[:], op=OP.mult)
    h = pool.tile([p, g], f32, tag=f"{tag}_hf")
    V.tensor_scalar(out=h[:], in0=t[:], scalar1=1.0, scalar2=1.0,
                    op0=OP.mult, op1=OP.add)
    return h


# atanh(sqrt(u))/sqrt(u) = 1 + u/3 + u^2/5 + u^3/7 + u^4/9
A_COEF = [1.0 / 3, 1.0 / 5, 1.0 / 7, 1.0 / 9]
# tanh(sqrt(v))/sqrt(v) = 1 - v/3 + 2v^2/15 - 17v^3/315
T_COEF = [-1.0 / 3, 2.0 / 15, -17.0 / 315]


def _scaled(coeffs, s):
    return [c * (s ** (i + 1)) for i, c in enumerate(coeffs)]


def _fold_chain(nc, ch, S2, y2, alpha, nA2, g):
    """Mobius fold scalars: given S2=2<A,y>, y2, state (alpha,nA2) ->
    (beta, alpha', nA2'). All [P, g] fp32."""
    V = nc.vector
    t1 = ch.tile([P, g], f32, tag="fc_t1")  # 2 a d
    V.tensor_tensor(out=t1[:], in0=alpha[:], in1=S2[:], op=OP.mult)
    u1 = ch.tile([P, g], f32, tag="fc_u1")  # 2ad + y2
    V.tensor_tensor(out=u1[:], in0=t1[:], in1=y2[:], op=OP.add)
    aa = ch.tile([P, g], f32, tag="fc_aa")
    V.tensor_tensor(out=aa[:], in0=alpha[:], in1=alpha[:], op=OP.mult)
    a2 = ch.tile([P, g], f32, tag="fc_a2")
    V.tensor_tensor(out=a2[:], in0=aa[:], in1=nA2[:], op=OP.mult)
    t2 = ch.tile([P, g], f32, tag="fc_t2")
    V.tensor_tensor(out=t2[:], in0=a2[:], in1=y2[:], op=OP.mult)
    u2 = ch.tile([P, g], f32, tag="fc_u2")  # 2ad + a2*y2
    V.tensor_tensor(out=u2[:], in0=t1[:], in1=t2[:], op=OP.add)
    den = ch.tile([P, g], f32, tag="fc_den")  # 1 + 2ad + a2*y2
    V.tensor_scalar(out=den[:], in0=u2[:], scalar1=1.0, scalar2=None,
                    op0=OP.add)
    ca = ch.tile([P, g], f32, tag="fc_ca")   # 1 + 2ad + y2
    V.tensor_scalar(out=ca[:], in0=u1[:], scalar1=1.0, scalar2=None,
                    op0=OP.add)
    r = ch.tile([P, g], f32, tag="fc_r")
    V.reciprocal(r[:], den[:])
    t3 = ch.tile([P, g], f32, tag="fc_t3")   # alpha*ca
    V.tensor_tensor(out=t3[:], in0=alpha[:], in1=ca[:], op=OP.mult)
    alpha_n = ch.tile([P, g], f32, tag="fc_an")
    V.tensor_tensor(out=alpha_n[:], in0=t3[:], in1=r[:], op=OP.mult)
    w = ch.tile([P, g], f32, tag="fc_w")     # 1 - a2
    V.tensor_scalar(out=w[:], in0=a2[:], scalar1=-1.0, scalar2=1.0,
                    op0=OP.mult, op1=OP.add)
    vinv = ch.tile([P, g], f32, tag="fc_vi")
    V.reciprocal(vinv[:], t3[:])
    beta = ch.tile([P, g], f32, tag="fc_be")
    V.tensor_tensor(out=beta[:], in0=w[:], in1=vinv[:], op=OP.mult)
    t4 = ch.tile([P, g], f32, tag="fc_t4")  # beta * 2d
    V.tensor_tensor(out=t4[:], in0=beta[:], in1=S2[:], op=OP.mult)
    t5 = ch.tile([P, g], f32, tag="fc_t5")   # nA2 + 2 beta d
    V.tensor_tensor(out=t5[:], in0=t4[:], in1=nA2[:], op=OP.add)
    t6 = ch.tile([P, g], f32, tag="fc_t6")
    V.tensor_tensor(out=t6[:], in0=beta[:], in1=beta[:], op=OP.mult)
    t7 = ch.tile([P, g], f32, tag="fc_t7")
    V.tensor_tensor(out=t7[:], in0=t6[:], in1=y2[:], op=OP.mult)
    nA2_n = ch.tile([P, g], f32, tag="fc_nn")
    V.tensor_tensor(out=nA2_n[:], in0=t5[:], in1=t7[:], op=OP.add)
    return beta, alpha_n, nA2_n


def _acc_row(acc, t):
    return acc[:, t, 0:DH]


def _build_nc():
    nc = bacc.Bacc("TRN2", target_bir_lowering=False, debug=False,
                   num_devices=NCORES, num_swdge_queues=NQ)
    featT = nc.dram_tensor("featT", [DIN, NSH], bf16, kind="ExternalInput")
    featR = nc.dram_tensor("featR", [NSH, DIN], bf16, kind="ExternalInput")
    idx = nc.dram_tensor("idx", [P, NG * KDEG * NCALL * IBS], i16,
                         kind="ExternalInput")
    sel = nc.dram_tensor("sel", [8, 1], i32, kind="ExternalInput")
    W1 = nc.dram_tensor("W1", [DIN, DH], bf16, kind="ExternalInput")
    b1 = nc.dram_tensor("b1", [1, DH], f16, kind="ExternalInput")
    W2 = nc.dram_tensor("W2", [DH, DH], f16, kind="ExternalInput")
    b2 = nc.dram_tensor("b2", [1, DH], f16, kind="ExternalInput")
    WlT = nc.dram_tensor("WlT", [DH, DOUT], f16, kind="ExternalInput")
    bl = nc.dram_tensor("bl", [1, DOUT], f32, kind="ExternalInput")
    out = nc.dram_tensor("out", [8, DOUT], f32, kind="ExternalOutput")
    if KDEBUG:
        dbg_t1 = nc.dram_tensor("dbg_t1", [NSH, 264], f16,
                                kind="ExternalOutput")
        dbg_h1 = nc.dram_tensor("dbg_h1", [NSH, 264], f16,
                                kind="ExternalOutput")
        dbg_h2 = nc.dram_tensor("dbg_h2", [NSH, 264], f16,
                                kind="ExternalOutput")

    with tile.TileContext(nc) as tc:
        with (
            tc.tile_pool(name="wt", bufs=1) as wt,
            tc.tile_pool(name="sb", bufs=3) as sb,
            tc.tile_pool(name="ch", bufs=2) as ch,
            tc.tile_pool(name="yb", bufs=2) as yb,
            tc.tile_pool(name="ac", bufs=1) as ac,
            tc.tile_pool(name="df", bufs=2) as df,
            tc.tile_pool(name="tp", bufs=1) as tp,
            tc.tile_pool(name="ps", bufs=2, space="PSUM") as ps,
            tc.tile_pool(name="psmx", bufs=4, space="PSUM") as psmx,
            tc.tile_pool(name="psm2", bufs=2, space="PSUM") as psm2,
            tc.tile_pool(name="dr", bufs=1, space="DRAM") as dr,
        ):
            ident = wt.tile([P, P], f16, tag="ident")
            make_identity(nc, ident[:])

            # ---- weights ----
            W1sb = wt.tile([P, DIN // P, DH], bf16, tag="W1sb")
            nc.sync.dma_start(W1sb[:],
                              W1[:].rearrange("(a p) d -> p a d", p=P))
            W2sb = wt.tile([P, DH // P, DH], f16, tag="W2sb")
            nc.sync.dma_start(W2sb[:],
                              W2[:].rearrange("(a p) d -> p a d", p=P))
            Wlsb = wt.tile([P, DH // P, DOUT], f16, tag="Wlsb")
            nc.sync.dma_start(Wlsb[:],
                              WlT[:].rearrange("(a p) d -> p a d", p=P))
            b1row = wt.tile([1, DH], f16, tag="b1row")
            nc.sync.dma_start(b1row[:], b1[:])
            b1b = wt.tile([P, DH], f16, tag="b1b")
            nc.gpsimd.partition_broadcast(b1b[:], b1row[:])
            b2row = wt.tile([1, DH], f16, tag="b2row")
            nc.sync.dma_start(b2row[:], b2[:])
            b2b = wt.tile([P, DH], f16, tag="b2b")
            nc.gpsimd.partition_broadcast(b2b[:], b2row[:])
            blrow = wt.tile([1, DOUT], f32, tag="blrow")
            nc.sync.dma_start(blrow[:], bl[:])
            blb = wt.tile([8, DOUT], f32, tag="blb")
            nc.gpsimd.partition_broadcast(blb[:], blrow[:], channels=8)
            bscr = wt.tile([P, DH], f16, tag="bscr")
            b1n2 = wt.tile([P, 1], f32, tag="b1n2")
            nc.scalar.activation(bscr[:], b1b[:], AF.Square, accum_out=b1n2[:])
            bscr2 = wt.tile([P, DH], f16, tag="bscr2")
            b2n2 = wt.tile([P, 1], f32, tag="b2n2")
            nc.scalar.activation(bscr2[:], b2b[:], AF.Square,
                                 accum_out=b2n2[:])
            bscr3 = wt.tile([8, DOUT], f32, tag="bscr3")
            bln2 = wt.tile([8, 1], f32, tag="bln2")
            nc.scalar.activation(bscr3[:], blb[:], AF.Square,
                                 accum_out=bln2[:])

            # ---- indices (biased int16, replicated over partitions) ----
            idx_sb = wt.tile([P, NG * KDEG * NCALL * IBS], i16, tag="idx")
            nc.sync.dma_start(idx_sb[:], idx[:])

            def idx_slice(gi, k, j):
                o = (((gi * KDEG) + k) * NCALL + j) * IBS
                return idx_sb[:, o : o + IB]

            # ---- DRAM tables ----
            t1sh = dr.tile([NSH, SE], f16, tag="t1sh")
            t1full = dr.tile([N, SE], f16, tag="t1full", addr_space="Shared")
            h1sh = dr.tile([NSH, SE], f16, tag="h1sh")
            h1full = dr.tile([N, SE], f16, tag="h1full", addr_space="Shared")
            h2sh = dr.tile([NSH, SE], f16, tag="h2sh")

            # ============ Phase T: layer-1 transform ============
            cc_insts_1 = []
            for bt in (range(0, TILES, TG) if STAGE >= 1 else []):
                xn2 = ch.tile([P, TG], f32, tag="tf_xn2")
                mxn2 = ch.tile([P, TG], f32, tag="tf_mxn2")
                pmx_list = []
                for j in range(TG):
                    t = bt + j
                    ftT = sb.tile([P, DIN // P, P], bf16, tag="ftT")
                    nc.sync.dma_start(
                        ftT[:],
                        featT[:, t * P : (t + 1) * P].rearrange(
                            "(c p) n -> p c n", p=P))
                    ftR = sb.tile([P, DIN], bf16, tag="ftR")
                    nc.sync.dma_start(ftR[:], featR[t * P : (t + 1) * P, :])
                    sq = sb.tile([P, DIN], bf16, tag="sq")
                    nc.scalar.activation(sq[:], ftR[:], AF.Square,
                                         accum_out=xn2[:, j : j + 1])
                    pmx = psmx.tile([P, DH], f32, tag="pmx")
                    for c in range(DIN // P):
                        nc.tensor.matmul(out=pmx[:], lhsT=ftT[:, c, :],
                                         rhs=W1sb[:, c, :],
                                         start=(c == 0),
                                         stop=(c == DIN // P - 1))
                    msq = sb.tile([P, DH], f16, tag="msq")
                    nc.scalar.activation(msq[:], pmx[:], AF.Square,
                                         accum_out=mxn2[:, j : j + 1])
                    pmx_list.append(pmx)
                # batched scalar chain: s=poly_a(xn2/16), r2=(mxn2/16)*s^2
                s = _poly_horner(nc, ch, xn2, _scaled(A_COEF, NORM2), TG,
                                 "tfa")
                ssq = ch.tile([P, TG], f32, tag="tf_ssq")
                nc.vector.tensor_tensor(out=ssq[:], in0=s[:], in1=s[:],
                                        op=OP.mult)
                mxs = ch.tile([P, TG], f32, tag="tf_mxs")
                nc.vector.tensor_tensor(out=mxs[:], in0=mxn2[:], in1=ssq[:],
                                        op=OP.mult)
                pt = _poly_horner(nc, ch, mxs, _scaled(T_COEF, NORM2), TG,
                                  "tft")
                spt = ch.tile([P, TG], f32, tag="tf_spt")
                nc.vector.tensor_tensor(out=spt[:], in0=s[:], in1=pt[:],
                                        op=OP.mult)
                coef = ch.tile([P, TG], f32, tag="tf_coef")
                nc.vector.tensor_scalar(out=coef[:], in0=spt[:],
                                        scalar1=NORM, scalar2=None,
                                        op0=OP.mult)
                # y2 = (r*pt)^2 = (mxs/16)*pt^2 * s^2... careful:
                # row = coef*pmx; y2 = coef^2 * mxn2
                c2 = ch.tile([P, TG], f32, tag="tf_c2")
                nc.vector.tensor_tensor(out=c2[:], in0=coef[:], in1=coef[:],
                                        op=OP.mult)
                y2r = ch.tile([P, TG], f32, tag="tf_y2r")
                nc.vector.tensor_tensor(out=y2r[:], in0=c2[:], in1=mxn2[:],
                                        op=OP.mult)
                for j in range(TG):
                    t = bt + j
                    htile = sb.tile([P, 257], f16, tag="htile")
                    nc.vector.tensor_scalar(
                        out=htile[:, 0:DH], in0=pmx_list[j][:],
                        scalar1=coef[:, j : j + 1], scalar2=None, op0=OP.mult)
                    nc.vector.tensor_copy(out=htile[:, DH : DH + 1],
                                          in_=y2r[:, j : j + 1])
                    nc.sync.dma_start(t1sh[t * P : (t + 1) * P, 0:257],
                                      htile[:])

            if STAGE >= 2:
                cc = nc.gpsimd.collective_compute(
                    "AllGather", OP.bypass,
                    replica_groups=[list(range(NCORES))],
                    ins=[t1sh[:]], outs=[t1full[:]])
                cc_insts_1.append(cc)

            # ============ fold machinery ============
            def fold_layer(table, cc_insts, bias_b, bias_n2, matvec2,
                           dst_sh):
                """One aggregation layer over `table` (gather source).

                matvec2: if True apply W2 mobius matvec after the fold
                (layer 2); else layer-1 style (store with NORM scale).
                """
                base = table[N // 2 :, :]
                for gi in range(NG):
                    acc = ac.tile([P, G, DH], f16, tag="acc")
                    gaths = []
                    ybuf0 = yb.tile([P, NCALL, TPQ + 1, SE], f16, tag="ybuf")
                    for j in range(NCALL):
                        g0 = nc.gpsimd.dma_gather(
                            out_ap=ybuf0[:, j, :, :], in_ap=base,
                            idxs_ap=idx_slice(gi, 0, j),
                            num_idxs=NI, num_idxs_reg=NI, elem_size=SE,
                            single_packet=False, queue_num=j % NQ)
                        gaths.append(g0)
                    nc.vector.tensor_copy(
                        out=acc[:].rearrange("p (q b) c -> p q b c", q=NCALL),
                        in_=ybuf0[:, :, 0:TPQ, 0:DH])
                    alpha = ch.tile([P, G], f32, tag="st_alpha")
                    nc.vector.memset(alpha[:], 1.0)
                    nA2 = ch.tile([P, G], f32, tag="st_nA2")
                    nc.vector.tensor_copy(
                        out=nA2[:].rearrange("p (q b) -> p q b", q=NCALL),
                        in_=ybuf0[:, :, 0:TPQ, DH])
                    scratch = sb.tile([P, DH], f16, tag="dotscr")

                    for k in range(1, KDEG):
                        ybuf = yb.tile([P, NCALL, TPQ + 1, SE], f16,
                                       tag="ybuf")
                        for j in range(NCALL):
                            gk = nc.gpsimd.dma_gather(
                                out_ap=ybuf[:, j, :, :], in_ap=base,
                                idxs_ap=idx_slice(gi, k, j),
                                num_idxs=NI, num_idxs_reg=NI, elem_size=SE,
                                single_packet=False, queue_num=j % NQ)
                            gaths.append(gk)
                        prod = df.tile([P, G, DH], bf16, tag="prod")
                        nc.vector.tensor_tensor(
                            out=prod[:].rearrange("p (q b) c -> p q b c",
                                                  q=NCALL),
                            in0=acc[:].rearrange("p (q b) c -> p q b c",
                                                 q=NCALL),
                            in1=ybuf[:, :, 0:TPQ, 0:DH], op=OP.mult)
                        red = df.tile([P, G, DH // 2], bf16, tag="red")
                        nc.vector.tensor_tensor(
                            out=red[:], in0=prod[:, :, 0 : DH // 2],
                            in1=prod[:, :, DH // 2 : DH], op=OP.add)
                        w = DH // 2
                        while w > 4:
                            nc.vector.tensor_tensor(
                                out=red[:, :, 0 : w // 2],
                                in0=red[:, :, 0 : w // 2],
                                in1=red[:, :, w // 2 : w], op=OP.add)
                            w //= 2
                        d = ch.tile([P, G], f32, tag="fd_d")
                        nc.vector.tensor_reduce(
                            out=d[:], in_=red[:, :, 0:4],
                            axis=mybir.AxisListType.X, op=OP.add)
                        y2 = ch.tile([P, G], f32, tag="fd_y2")
                        nc.vector.tensor_copy(
                            out=y2[:].rearrange("p (q b) -> p q b", q=NCALL),
                            in_=ybuf[:, :, 0:TPQ, DH])
                        S2 = ch.tile([P, G], f32, tag="fd_S2")
                        nc.vector.tensor_scalar(out=S2[:], in0=d[:],
                                                scalar1=2.0, scalar2=None,
                                                op0=OP.mult)
                        beta, alpha, nA2 = _fold_chain(nc, ch, S2, y2,
                                                       alpha, nA2, G)
                        b16 = ch.tile([P, G], f16, tag="fd_b16")
                        nc.vector.tensor_copy(out=b16[:], in_=beta[:])
                        bbc = b16[:].unsqueeze(2).to_broadcast([P, G, DH])
                        tmp = tp.tile([P, G, DH], f16, tag="tmp")
                        nc.vector.tensor_tensor(
                            out=tmp[:].rearrange("p (q b) c -> p q b c",
                                                 q=NCALL),
                            in0=ybuf[:, :, 0:TPQ, 0:DH],
                            in1=bbc.rearrange("p (q b) c -> p q b c",
                                              q=NCALL), op=OP.mult)
                        if int(os.environ.get("KDMAADD", "1")):
                            for u in range(4):
                                nc.gpsimd.dma_start(
                                    acc[:, u * 8 : (u + 1) * 8, :],
                                    tmp[:, u * 8 : (u + 1) * 8, :],
                                    accum_op=OP.add)
                        else:
                            nc.vector.tensor_tensor(out=acc[:], in0=acc[:],
                                                    in1=tmp[:], op=OP.add)
                    for gth in gaths:
                        for cci in cc_insts:
                            add_dep_helper(gth.ins, cci.ins,
                                           reason="gather after allgather")

                    if matvec2:
                        # x = alpha*A; mx = alpha*(A @ W2); v-rows = pmA
                        a2 = ch.tile([P, G], f32, tag="mv_a2")
                        nc.vector.tensor_tensor(out=a2[:], in0=alpha[:],
                                                in1=alpha[:], op=OP.mult)
                        nc.vector.tensor_tensor(out=a2[:], in0=a2[:],
                                                in1=nA2[:], op=OP.mult)
                        s2g = _poly_horner(nc, ch, a2, A_COEF, G, "mva")
                        msqA = ch.tile([P, G], f32, tag="mv_msq")
                        for t in range(G):
                            aT = sb.tile([P, DH // P, P], f16, tag="aT")
                            for c in range(DH // P):
                                ptr = ps.tile([P, P], f16, tag="ptr")
                                nc.tensor.transpose(
                                    out=ptr[:],
                                    in_=acc[:, t, c * P : (c + 1) * P],
                                    identity=ident[:])
                                nc.vector.tensor_copy(out=aT[:, c, :],
                                                      in_=ptr[:])
                            pm2 = psm2.tile([P, DH], f32, tag="pm2")
                            for c in range(DH // P):
                                nc.tensor.matmul(out=pm2[:],
                                                 lhsT=aT[:, c, :],
                                                 rhs=W2sb[:, c, :],
                                                 start=(c == 0),
                                                 stop=(c == DH // P - 1))
                            ms2 = sb.tile([P, DH], f16, tag="ms2")
                            nc.scalar.activation(ms2[:], pm2[:], AF.Square,
                                                 accum_out=msqA[:, t : t + 1])
                            # overwrite acc row with pmA (fp16)
                            nc.vector.tensor_copy(
                                out=acc[:, t, 0:DH], in_=pm2[:])
                        # r2 = a2 * msqA * s^2; coef = alpha*s*pt(r2)*NORM
                        ssq = ch.tile([P, G], f32, tag="mv_ssq")
                        nc.vector.tensor_tensor(out=ssq[:], in0=s2g[:],
                                                in1=s2g[:], op=OP.mult)
                        aa2 = ch.tile([P, G], f32, tag="mv_aa2")
                        nc.vector.tensor_tensor(out=aa2[:], in0=alpha[:],
                                                in1=alpha[:], op=OP.mult)
                        r2 = ch.tile([P, G], f32, tag="mv_r2")
                        nc.vector.tensor_tensor(out=r2[:], in0=aa2[:],
                                                in1=msqA[:], op=OP.mult)
                        nc.vector.tensor_tensor(out=r2[:], in0=r2[:],
                                                in1=ssq[:], op=OP.mult)
                        ptg = _poly_horner(nc, ch, r2, T_COEF, G, "mvt")
                        co = ch.tile([P, G], f32, tag="mv_co")
                        nc.vector.tensor_tensor(out=co[:], in0=alpha[:],
                                                in1=s2g[:], op=OP.mult)
                        nc.vector.tensor_tensor(out=co[:], in0=co[:],
                                                in1=ptg[:], op=OP.mult)
                        alpha = ch.tile([P, G], f32, tag="st_alpha")
                        nc.vector.tensor_scalar(out=alpha[:], in0=co[:],
                                                scalar1=NORM, scalar2=None,
                                                op0=OP.mult)
                        nA2 = msqA
                        bias_row, bias_sq = b2b, b2n2
                        scale = 1.0
                    else:
                        alpha_s = ch.tile([P, G], f32, tag="st_alpha")
                        nc.vector.tensor_scalar(out=alpha_s[:], in0=alpha[:],
                                                scalar1=NORM, scalar2=None,
                                                op0=OP.mult)
                        alpha = alpha_s
                        bias_row, bias_sq = bias_b, bias_n2
                        scale = NORM

                    # bias fold: y = bias_row (broadcast), y2 = bias_sq
                    d = ch.tile([P, G], f32, tag="fd_d")
                    for t in range(G):
                        nc.vector.scalar_tensor_tensor(
                            out=scratch[:], in0=_acc_row(acc, t), scalar=1.0,
                            in1=bias_row[:], op0=OP.mult, op1=OP.mult,
                            accum_out=d[:, t : t + 1])
                    S2b = ch.tile([P, G], f32, tag="fd_S2b")
                    nc.vector.tensor_scalar(out=S2b[:], in0=d[:], scalar1=2.0,
                                            scalar2=None, op0=OP.mult)
                    y2b = ch.tile([P, G], f32, tag="fd_y2b")
                    nc.vector.tensor_scalar(out=y2b[:], in0=d[:], scalar1=0.0,
                                            scalar2=bias_sq[:, 0:1],
                                            op0=OP.mult, op1=OP.add)
                    beta, alpha, nA2 = _fold_chain(nc, ch, S2b, y2b, alpha,
                                                   nA2, G)
                    for t in range(G):
                        nc.vector.scalar_tensor_tensor(
                            out=_acc_row(acc, t), in0=bias_row[:],
                            scalar=beta[:, t : t + 1], in1=_acc_row(acc, t),
                            op0=OP.mult, op1=OP.add)

                    # activation: logmap0 -> relu -> expmap0, store
                    rn2g = ch.tile([P, G], f32, tag="as_rn2")
                    for t in range(G):
                        zrow = _acc_row(acc, t)
                        nc.vector.tensor_scalar(out=zrow, in0=zrow,
                                                scalar1=0.0, scalar2=None,
                                                op0=OP.max)
                        zsq = sb.tile([P, DH], f16, tag="zsq")
                        nc.scalar.activation(zsq[:], zrow, AF.Square,
                                             accum_out=rn2g[:, t : t + 1])
                    a2f = ch.tile([P, G], f32, tag="as_a2f")
                    nc.vector.tensor_tensor(out=a2f[:], in0=alpha[:],
                                            in1=alpha[:], op=OP.mult)
                    nc.vector.tensor_tensor(out=a2f[:], in0=a2f[:],
                                            in1=nA2[:], op=OP.mult)
                    sg = _poly_horner(nc, ch, a2f, A_COEF, G, "asa")
                    c1 = ch.tile([P, G], f32, tag="as_c1")
                    nc.vector.tensor_tensor(out=c1[:], in0=sg[:],
                                            in1=alpha[:], op=OP.mult)
                    c1sq = ch.tile([P, G], f32, tag="as_c1sq")
                    nc.vector.tensor_tensor(out=c1sq[:], in0=c1[:],
                                            in1=c1[:], op=OP.mult)
                    vr2 = ch.tile([P, G], f32, tag="as_vr2")
                    nc.vector.tensor_tensor(out=vr2[:], in0=c1sq[:],
                                            in1=rn2g[:], op=OP.mult)
                    ptg2 = _poly_horner(nc, ch, vr2, T_COEF, G, "ast")
                    gam = ch.tile([P, G], f32, tag="as_gam")
                    nc.vector.tensor_tensor(out=gam[:], in0=c1[:],
                                            in1=ptg2[:], op=OP.mult)
                    if scale != 1.0:
                        gs = ch.tile([P, G], f32, tag="as_gs")
                        nc.vector.tensor_scalar(out=gs[:], in0=gam[:],
                                                scalar1=scale, scalar2=None,
                                                op0=OP.mult)
                        gam = gs
                    gg = ch.tile([P, G], f32, tag="as_gg")
                    nc.vector.tensor_tensor(out=gg[:], in0=gam[:],
                                            in1=gam[:], op=OP.mult)
                    y2o = ch.tile([P, G], f32, tag="as_y2o")
                    nc.vector.tensor_tensor(out=y2o[:], in0=gg[:],
                                            in1=rn2g[:], op=OP.mult)
                    for t in range(G):
                        ti = gi * G + t
                        htile = sb.tile([P, 257], f16, tag="htile")
                        nc.vector.tensor_scalar(
                            out=htile[:, 0:DH], in0=_acc_row(acc, t),
                            scalar1=gam[:, t : t + 1], scalar2=None,
                            op0=OP.mult)
                        nc.vector.tensor_copy(out=htile[:, DH : DH + 1],
                                              in_=y2o[:, t : t + 1])
                        nc.sync.dma_start(
                            dst_sh[ti * P : (ti + 1) * P, 0:257], htile[:])
                return

            # ============ Phase A1 + AllGather h1 ============
            cc_insts_2 = []
            if STAGE >= 3:
                fold_layer(t1full, cc_insts_1, b1b, b1n2, False, h1sh)
                if STAGE >= 4:
                    cc = nc.gpsimd.collective_compute(
                        "AllGather", OP.bypass,
                        replica_groups=[list(range(NCORES))],
                        ins=[h1sh[:]], outs=[h1full[:]])
                    cc_insts_2.append(cc)

            # ============ Phase A2 ============
            if STAGE >= 4:
                fold_layer(h1full, cc_insts_2, b2b, b2n2, True, h2sh)

            # ============ Phase D: classifier ============
            if STAGE >= 5:
                selt = wt.tile([8, 1], i32, tag="selt")
                nc.sync.dma_start(selt[:], sel[:])
                hr = wt.tile([8, SE], f16, tag="hr")
                nc.gpsimd.indirect_dma_start(
                    out=hr[:], out_offset=None, in_=h2sh[:],
                    in_offset=bass.IndirectOffsetOnAxis(ap=selt[:, 0:1],
                                                        axis=0))
                hT = wt.tile([P, DH // P, 8], f16, tag="hT")
                for c in range(DH // P):
                    pt3 = ps.tile([P, P], f16, tag="ptr")
                    nc.tensor.transpose(out=pt3[:, 0:8],
                                        in_=hr[:, c * P : (c + 1) * P],
                                        identity=ident[0:8, 0:8])
                    nc.vector.tensor_copy(out=hT[:, c, :], in_=pt3[:, 0:8])
                pmf = psm2.tile([8, DOUT], f32, tag="pm2")
                for c in range(DH // P):
                    nc.tensor.matmul(out=pmf[:], lhsT=hT[:, c, :],
                                     rhs=Wlsb[:, c, :],
                                     start=(c == 0), stop=(c == DH // P - 1))
                mfn2 = wt.tile([8, 1], f32, tag="mfn2")
                msf = wt.tile([8, DOUT], f32, tag="msf")
                nc.scalar.activation(msf[:], pmf[:], AF.Square,
                                     accum_out=mfn2[:])
                xnf2 = wt.tile([8, 1], f32, tag="xnf2")
                nc.vector.tensor_copy(out=xnf2[:], in_=hr[:, DH : DH + 1])
                sf = _poly_horner(nc, wt, xnf2, A_COEF, 1, "fna", p=8)
                ssf = wt.tile([8, 1], f32, tag="fn_ssq")
                nc.vector.tensor_tensor(out=ssf[:], in0=sf[:], in1=sf[:],
                                        op=OP.mult)
                rf2 = wt.tile([8, 1], f32, tag="fn_r2")
                nc.vector.tensor_tensor(out=rf2[:], in0=mfn2[:], in1=ssf[:],
                                        op=OP.mult)
                ptf = _poly_horner(nc, wt, rf2, T_COEF, 1, "fnt", p=8)
                cof = wt.tile([8, 1], f32, tag="fn_co")
                nc.vector.tensor_tensor(out=cof[:], in0=sf[:], in1=ptf[:],
                                        op=OP.mult)
                Vf = wt.tile([8, DOUT], f32, tag="Vf")
                nc.vector.tensor_scalar(out=Vf[:], in0=pmf[:],
                                        scalar1=cof[:], scalar2=None,
                                        op0=OP.mult)
                x2f = wt.tile([8, 1], f32, tag="x2f")  # ||Vf||^2
                nc.vector.tensor_tensor(out=x2f[:], in0=cof[:], in1=cof[:],
                                        op=OP.mult)
                nc.vector.tensor_tensor(out=x2f[:], in0=x2f[:], in1=mfn2[:],
                                        op=OP.mult)
                dotf = wt.tile([8, 1], f32, tag="dotf")
                prodf = wt.tile([8, DOUT], f32, tag="prodf")
                nc.vector.scalar_tensor_tensor(
                    out=prodf[:], in0=Vf[:], scalar=1.0, in1=blb[:],
                    op0=OP.mult, op1=OP.mult, accum_out=dotf[:])
                u1f = wt.tile([8, 1], f32, tag="u1f")  # 2d + bln2
                nc.vector.scalar_tensor_tensor(out=u1f[:], in0=dotf[:],
                                               scalar=2.0, in1=bln2[:],
                                               op0=OP.mult, op1=OP.add)
                t2f = wt.tile([8, 1], f32, tag="t2f")
                nc.vector.tensor_tensor(out=t2f[:], in0=x2f[:], in1=bln2[:],
                                        op=OP.mult)
                denf = wt.tile([8, 1], f32, tag="denf")
                nc.vector.scalar_tensor_tensor(out=denf[:], in0=t2f[:],
                                               scalar=1.0, in1=u1f[:],
                                               op0=OP.add, op1=OP.add)
                rdf = wt.tile([8, 1], f32, tag="rdf")
                nc.vector.reciprocal(rdf[:], denf[:])
                caf = wt.tile([8, 1], f32, tag="caf")
                nc.vector.tensor_scalar(out=caf[:], in0=u1f[:], scalar1=1.0,
                                        scalar2=None, op0=OP.add)
                s1f = wt.tile([8, 1], f32, tag="s1f")
                nc.vector.tensor_tensor(out=s1f[:], in0=caf[:], in1=rdf[:],
                                        op=OP.mult)
                wf = wt.tile([8, 1], f32, tag="wf")
                nc.vector.tensor_scalar(out=wf[:], in0=x2f[:], scalar1=-1.0,
                                        scalar2=1.0, op0=OP.mult, op1=OP.add)
                s2f = wt.tile([8, 1], f32, tag="s2f")
                nc.vector.tensor_tensor(out=s2f[:], in0=wf[:], in1=rdf[:],
                                        op=OP.mult)
                vs1 = wt.tile([8, DOUT], f32, tag="vs1")
                nc.vector.tensor_scalar(out=vs1[:], in0=Vf[:],
                                        scalar1=s1f[:], scalar2=None,
                                        op0=OP.mult)
                outt = wt.tile([8, DOUT], f32, tag="outt")
                nc.vector.scalar_tensor_tensor(
                    out=outt[:], in0=blb[:], scalar=s2f[:], in1=vs1[:],
                    op0=OP.mult, op1=OP.add)
                nc.sync.dma_start(out[:], outt[:])
            else:
                dumt = wt.tile([8, DOUT], f32, tag="dumt")
                nc.vector.memset(dumt[:], 0.0)
                nc.sync.dma_start(out[:], dumt[:])
            if KDEBUG:
                nc.sync.dma_start(dbg_t1[:], t1sh[:])
                if STAGE >= 3:
                    nc.sync.dma_start(dbg_h1[:], h1sh[:])
                if STAGE >= 4:
                    nc.sync.dma_start(dbg_h2[:], h2sh[:])

    nc.compile()
    return nc


def _get_nc():
    if "nc" not in _NC_CACHE:
        _NC_CACHE["nc"] = _build_nc()
    return _NC_CACHE["nc"]


def _prep_indices(src_idx):
    """Remap global node ids to chunk-gathered table rows, bias to int16,
    arrange per (core, group, k, queue) in dma_gather wrapped layout."""
    # unchunked AllGather: full-table row for node n is just n
    remap = np.arange(N, dtype=np.int64)

    idx_all = np.zeros((NCORES, P, NG * KDEG * NCALL * IBS), dtype=np.int16)
    for cre in range(NCORES):
        si = src_idx[cre * NSH : (cre + 1) * NSH]  # [NSH, K]
        r = remap[si] - 32768  # [NSH, K] int64 in [-32768, 32767]
        cols = []
        for gi in range(NG):
            for k in range(KDEG):
                for j in range(NCALL):
                    lo = gi * G * P + j * TPQ * P
                    lst = r[lo : lo + TPQ * P, k]  # order: idx i = blk*128+p
                    lst = np.concatenate(
                        [lst, np.zeros(NI - TPQ * P, dtype=np.int64)])
                    blk = lst.reshape(IB, 16).T.astype(np.int16)  # [16, IB]
                    blk = np.pad(blk, ((0, 0), (0, IBS - IB)))
                    cols.append(np.tile(blk, (8, 1)))  # [128, IBS]
        idx_all[cre] = np.concatenate(cols, axis=1)
    return idx_all


def kernel(features, W1, b1, W2, b2, Wl, bl, src_idx, to_fetch):
    global LAST_EXEC_NS, LAST_RESULT
    nc = _get_nc()
    features = np.asarray(features, dtype=np.float32)
    src_idx = np.asarray(src_idx, dtype=np.int64)
    to_fetch = np.asarray(to_fetch, dtype=np.int64)
    import ml_dtypes
    W1b = np.ascontiguousarray(np.asarray(W1)).astype(ml_dtypes.bfloat16)
    b1h = np.asarray(b1, np.float16).reshape(1, DH)
    W2h = np.ascontiguousarray(np.asarray(W2)).astype(np.float16)
    b2h = np.asarray(b2, np.float16).reshape(1, DH)
    WlTh = np.ascontiguousarray(np.asarray(Wl).T).astype(np.float16)
    blf = np.asarray(bl, np.float32).reshape(1, DOUT)

    idx_all = _prep_indices(src_idx)

    in_maps = []
    for c in range(NCORES):
        fsh = features[c * NSH : (c + 1) * NSH]
        featR = np.ascontiguousarray(fsh).astype(ml_dtypes.bfloat16)
        featT = np.ascontiguousarray(fsh.T).astype(ml_dtypes.bfloat16)
        bidx = np.arange(c * 8, (c + 1) * 8, dtype=np.int64)
        selv = (to_fetch[bidx] + bidx * (N // 64) - c * NSH).astype(
            np.int32).reshape(8, 1)
        in_maps.append({
            "featT": featT, "featR": featR, "idx": idx_all[c], "sel": selv,
            "W1": W1b, "b1": b1h, "W2": W2h, "b2": b2h, "WlT": WlTh,
            "bl": blf,
        })
    res = run_bass_kernel_spmd(nc, in_maps, core_ids=list(range(NCORES)),
                               trace=TRACE)
    LAST_RESULT = res
    LAST_EXEC_NS = res.exec_time_ns
    return np.concatenate([res.results[c]["out"] for c in range(NCORES)],
                          axis=0)
